# revision 1
# baseline (speedup 1.0000x reference)
"""DFlash Qwen3 cross-attention on 8 TRN2 NeuronCores.

Sharding: tensor-parallel over heads. Core c owns KV head c (KVH=8) and the
4 query heads 4c..4c+3 of its GQA group. Each core computes its heads'
QKV projections, per-head RMSNorm + RoPE, causal attention, then the
normalized per-head attention outputs (laid out transposed, [4*D, QL]) are
AllGathered so every core holds attn^T [H*D, QL]; each core then computes a
512-column slice of o_proj and the host concatenates the 8 slices.

All matmuls run in bf16 (fp32 PSUM accumulation); softmax in fp32.
Host-side prep: transpose ck=concat(context,query) to [HID, KV] bf16,
slice per-core weights, precompute RoPE cos/sin and causal mask tiles.
"""

from contextlib import ExitStack

import numpy as np
from ml_dtypes import bfloat16

import concourse.bass as bass
import concourse.bass_isa as bass_isa
import concourse.mybir as mybir
import concourse.tile as tile
from concourse import bacc
from concourse.bass_utils import run_bass_kernel_spmd
from concourse.masks import make_identity

H = 32
KVH = 8
D = 128
HID = 4096
CTX = 4096
QL = 2048
KV = CTX + QL  # 6144
NCORES = 8
HPC = H // NCORES  # 4 q heads per core
THETA = 1000000.0
EPS = 1e-6
SCALE = float(D) ** -0.5

NHD = HID // 128  # 32 contraction chunks
NKV = KV // 128  # 48 kv chunks
NQC = QL // 128  # 16 q row chunks
NQJ = QL // 512  # 4 q column tiles for attention
MASKVAL = -1e6

F32 = mybir.dt.float32
BF16 = mybir.dt.bfloat16

_STATE = {}


def _build():
    nc = bacc.Bacc()

    ckT = nc.declare_dram_parameter("ckT", [HID, KV], BF16, isOutput=False)
    wq = nc.declare_dram_parameter("wq", [HID, HPC * D], BF16, isOutput=False)
    wkv = nc.declare_dram_parameter("wkv", [HID, 2 * D], BF16, isOutput=False)
    wo = nc.declare_dram_parameter("wo", [HID, HPC * D], BF16, isOutput=False)
    cs = nc.declare_dram_parameter("cs", [KV, D], F32, isOutput=False)
    nw = nc.declare_dram_parameter("nw", [128, 2 * D], F32, isOutput=False)
    msk = nc.declare_dram_parameter("msk", [128, 4 * 512], F32, isOutput=False)
    out_ext = nc.declare_dram_parameter("out", [QL, HPC * D], F32, isOutput=True)

    # per-(head, q-tile) AllGather buffers: each collective fires as soon as
    # that j-tile of that head is normalized, overlapping remaining compute;
    # o_proj consumes them in the same fine-grained order.
    ag_ins = [[nc.dram_tensor(f"ag_in{h}_{j}", [D, 512], BF16) for j in range(NQJ)]
              for h in range(HPC)]
    ag_outs = [[nc.dram_tensor(f"ag_out{h}_{j}", [NCORES * D, 512], BF16,
                               addr_space="Shared") for j in range(NQJ)]
               for h in range(HPC)]

    with tile.TileContext(nc) as tc, ExitStack() as ctx:
        singles = ctx.enter_context(tc.tile_pool(name="singles", bufs=1))
        # streamed ckT slices for the projections
        ckq_pool = ctx.enter_context(tc.tile_pool(name="ckq", bufs=4))
        wqs_pool = ctx.enter_context(tc.tile_pool(name="wqs", bufs=3))
        cs_pool = ctx.enter_context(tc.tile_pool(name="csp", bufs=3))
        # fp32 evacuation + norm/rope working tiles
        evac_pool = ctx.enter_context(tc.tile_pool(name="evac", bufs=3))
        tmp_pool = ctx.enter_context(tc.tile_pool(name="tmp", bufs=4))
        # attention-side pools
        p_pool = ctx.enter_context(tc.tile_pool(name="pt", bufs=4))
        sacc_pool = ctx.enter_context(tc.tile_pool(name="sacc", bufs=4))
        stg_pool = ctx.enter_context(tc.tile_pool(name="stg", bufs=4))
        oproj_pool = ctx.enter_context(tc.tile_pool(name="oproj", bufs=3))
        # PSUM: 4 accumulator banks + 4 shared banks (S^T chunks / transposes)
        acc_psum = ctx.enter_context(tc.tile_pool(name="accp", bufs=4, space="PSUM"))
        st_psum = ctx.enter_context(tc.tile_pool(name="stp", bufs=4, space="PSUM"))

        # ---- resident tensors ----
        wkv_sb = singles.tile([128, NHD, 2 * D], BF16)
        nc.scalar.dma_start(out=wkv_sb[:], in_=wkv[:, :].rearrange("(k p) n -> p k n", p=128))
        wo_sb = singles.tile([128, NHD, HPC * D], BF16)
        nc.scalar.dma_start(out=wo_sb[:], in_=wo[:, :].rearrange("(k p) n -> p k n", p=128))
        nw_sb = singles.tile([128, 2 * D], F32)
        nc.scalar.dma_start(out=nw_sb[:], in_=nw[:, :])
        msk_sb = singles.tile([128, 4 * 512], F32)
        nc.scalar.dma_start(out=msk_sb[:], in_=msk[:, :])

        ident = singles.tile([128, 128], F32)
        make_identity(nc, ident)
        epst = singles.tile([128, 1], F32)
        nc.vector.memset(epst, EPS)
        zbias = singles.tile([128, 1], F32)
        nc.vector.memset(zbias, 0.0)
        ones_col = singles.tile([128, 1], F32)
        nc.vector.memset(ones_col, 1.0)
        ones_row = singles.tile([1, 128], F32)
        nc.vector.memset(ones_row, 1.0)

        # outputs of the projection phases (bufs=1: written once, read later)
        qT_sb = singles.tile([128, HPC, QL], BF16)  # Q^T per head: [d, h, q]
        kT_sb = singles.tile([128, KV], BF16)  # K^T: [d, kv]
        v_sb = singles.tile([128, NKV, D], BF16)  # V: [kv%128, r, d]

        def rmsnorm_rope(xh, nw_col, cst, ro):
            """xh: [128, 128] f32 (rows = positions), normalized+roped -> ro."""
            sq = tmp_pool.tile([128, D], F32, tag="sq")
            nc.vector.tensor_mul(sq, xh, xh)
            ssum = tmp_pool.tile([128, 1], F32, tag="ssum")
            nc.vector.tensor_reduce(ssum, sq, axis=mybir.AxisListType.X, op=mybir.AluOpType.add)
            # ssum := sqrt(mean + eps); then reciprocal -> 1/rms
            nc.scalar.activation(out=ssum, in_=ssum, func=mybir.ActivationFunctionType.Sqrt,
                                 bias=epst, scale=1.0 / D)
            nc.vector.reciprocal(ssum, ssum)
            nc.vector.tensor_scalar_mul(out=xh, in0=xh, scalar1=ssum)
            nc.vector.tensor_mul(xh, xh, nw_sb[:, nw_col * D:(nw_col + 1) * D])
            c1 = cst[:, 0:64]
            s1 = cst[:, 64:128]
            t1 = tmp_pool.tile([128, 64], F32, tag="t1")
            nc.vector.tensor_mul(ro[:, 0:64], xh[:, 0:64], c1)
            nc.vector.tensor_mul(t1, xh[:, 64:128], s1)
            nc.vector.tensor_sub(ro[:, 0:64], ro[:, 0:64], t1)
            t2 = tmp_pool.tile([128, 64], F32, tag="t1")
            nc.vector.tensor_mul(ro[:, 64:128], xh[:, 64:128], c1)
            nc.vector.tensor_mul(t2, xh[:, 0:64], s1)
            nc.vector.tensor_add(ro[:, 64:128], ro[:, 64:128], t2)

        # ---- Q projection (+norm+rope+transpose) ----
        for qg in range(4):  # groups of 4 q row-chunks
            pq = [acc_psum.tile([128, HPC * D], F32, tag="acc", name=f"pq{qg}_{i}") for i in range(4)]
            for k2 in range(NHD // 2):
                cqt = ckq_pool.tile([128, 2, 512], BF16, tag="ckq")
                nc.gpsimd.dma_start(
                    out=cqt,
                    in_=ckT[k2 * 256:(k2 + 1) * 256,
                            CTX + qg * 512: CTX + (qg + 1) * 512].rearrange(
                        "(two p) c -> p two c", p=128))
                wqt = wqs_pool.tile([128, 2, HPC * D], BF16, tag="wqs")
                nc.gpsimd.dma_start(
                    out=wqt,
                    in_=wq[k2 * 256:(k2 + 1) * 256, :].rearrange("(two p) c -> p two c", p=128))
                for two in range(2):
                    k = 2 * k2 + two
                    for q4 in range(4):
                        nc.tensor.matmul(pq[q4], lhsT=cqt[:, two, q4 * 128:(q4 + 1) * 128],
                                         rhs=wqt[:, two, :], start=(k == 0), stop=(k == NHD - 1))
            for q4 in range(4):
                qc = qg * 4 + q4
                qe = evac_pool.tile([128, HPC * D], F32, tag="evac")
                nc.scalar.copy(out=qe, in_=pq[q4])
                cst = cs_pool.tile([128, D], F32, tag="csp")
                nc.gpsimd.dma_start(out=cst, in_=cs[(CTX // 128 + qc) * 128:(CTX // 128 + qc + 1) * 128, :])
                for h in range(HPC):
                    ro = tmp_pool.tile([128, D], F32, tag="ro")
                    rmsnorm_rope(qe[:, h * D:(h + 1) * D], 0, cst, ro)
                    tp = st_psum.tile([128, 128], F32, tag="st")
                    nc.tensor.transpose(tp, ro, ident)
                    nc.scalar.copy(out=qT_sb[:, h, qc * 128:(qc + 1) * 128], in_=tp)

        # ---- K/V projection (+norm+rope; K transposed, V natural) ----
        for rg in range(NKV // 4):  # groups of 4 kv chunks
            pk = [acc_psum.tile([128, 2 * D], F32, tag="acc", name=f"pk{rg}_{i}") for i in range(4)]
            for k2 in range(NHD // 2):
                ckt = ckq_pool.tile([128, 2, 512], BF16, tag="ckq")
                nc.gpsimd.dma_start(
                    out=ckt,
                    in_=ckT[k2 * 256:(k2 + 1) * 256,
                            rg * 512:(rg + 1) * 512].rearrange("(two p) c -> p two c", p=128))
                for two in range(2):
                    k = 2 * k2 + two
                    for r4 in range(4):
                        nc.tensor.matmul(pk[r4], lhsT=ckt[:, two, r4 * 128:(r4 + 1) * 128],
                                         rhs=wkv_sb[:, k, :], start=(k == 0), stop=(k == NHD - 1))
            for r4 in range(4):
                r = rg * 4 + r4
                ke = evac_pool.tile([128, 2 * D], F32, tag="evac")
                nc.scalar.copy(out=ke, in_=pk[r4])
                cst = cs_pool.tile([128, D], F32, tag="csp")
                nc.gpsimd.dma_start(out=cst, in_=cs[r * 128:(r + 1) * 128, :])
                ro = tmp_pool.tile([128, D], F32, tag="ro")
                rmsnorm_rope(ke[:, 0:D], 1, cst, ro)
                tp = st_psum.tile([128, 128], F32, tag="st")
                nc.tensor.transpose(tp, ro, ident)
                nc.scalar.copy(out=kT_sb[:, r * 128:(r + 1) * 128], in_=tp)
                nc.vector.tensor_copy(out=v_sb[:, r, :], in_=ke[:, D:2 * D])

        # ---- attention, per local head ----
        # S^T orientation: [kv partitions, q free]; exp output IS P^T; PV with
        # V stationary gives out^T [d, q] directly.  q position of col q is
        # CTX+j*512+q; kv chunk r fully visible iff r<=31+4j, partial for
        # i=r-32-4j in 0..3, masked out beyond.
        for h in range(HPC):
            o_acc = [acc_psum.tile([128, 512], F32, tag="acc", name=f"oacc{h}_{i}") for i in range(NQJ)]
            saccs = [sacc_pool.tile([128, 512], F32, tag="sacc", name=f"sacc{h}_{i}") for i in range(NQJ)]

            def normalize_j(j):
                # softmax denominator, normalize, stage, and AllGather this
                # (head, j) tile immediately -- overlaps remaining compute.
                pr = sacc_pool.tile([128, 512], F32, tag="pr")
                nc.gpsimd.partition_all_reduce(pr, saccs[j], channels=128,
                                               reduce_op=bass_isa.ReduceOp.add)
                nc.vector.reciprocal(pr, pr)
                stg = stg_pool.tile([128, 512], BF16, tag="stg")
                nc.vector.tensor_mul(stg, o_acc[j], pr)
                nc.gpsimd.dma_start(out=ag_ins[h][j][:], in_=stg)
                nc.gpsimd.collective_compute(
                    "AllGather",
                    mybir.AluOpType.bypass,
                    ins=[ag_ins[h][j][:]],
                    outs=[ag_outs[h][j][:]],
                    replica_groups=[list(range(NCORES))],
                )

            for r in range(NKV):
                js = [j for j in range(NQJ) if r <= 35 + 4 * j]
                for j in js:
                    st = st_psum.tile([128, 512], F32, tag="st")
                    nc.tensor.matmul(st, lhsT=kT_sb[:, r * 128:(r + 1) * 128],
                                     rhs=qT_sb[:, h, j * 512:(j + 1) * 512],
                                     start=True, stop=True)
                    i = r - 32 - 4 * j
                    if i >= 0:
                        nc.vector.tensor_add(st, st, msk_sb[:, i * 512:(i + 1) * 512])
                    pt = p_pool.tile([128, 512], BF16, tag="pt")
                    nc.scalar.activation(out=pt, in_=st,
                                         func=mybir.ActivationFunctionType.Exp,
                                         bias=zbias, scale=SCALE)
                    if r == 0:
                        nc.vector.tensor_copy(out=saccs[j], in_=pt)
                    else:
                        nc.vector.tensor_add(saccs[j], saccs[j], pt)
                    nc.tensor.matmul(o_acc[j], lhsT=v_sb[:, r, :], rhs=pt,
                                     start=(r == 0), stop=(r == 35 + 4 * j or r == NKV - 1))
                for j in range(NQJ):
                    if r == (35 + 4 * j if j < NQJ - 1 else NKV - 1):
                        normalize_j(j)

        # ---- o_proj: out[:, c*512:(c+1)*512] = attn @ wo_c ----
        # attn^T global row (4*core + h_local)*128 + d lives in ag_outs[h] row
        # core*128 + d, so wo_sb chunk index is 4*core + h.  Heads outermost so
        # matmuls over heads 0..2 run while head 3's AllGather is in flight.
        for qc in range(NQC):
            po = acc_psum.tile([128, HPC * D], F32, tag="acc")
            jq, qo = qc // 4, (qc % 4) * 128
            for h in range(HPC):
                at = oproj_pool.tile([128, NCORES, 128], BF16, tag="at")
                nc.gpsimd.dma_start(
                    out=at,
                    in_=ag_outs[h][jq][:, qo:qo + 128].rearrange(
                        "(c p) q -> p c q", p=128))
                for ci in range(NCORES):
                    nc.tensor.matmul(po, lhsT=at[:, ci, :], rhs=wo_sb[:, 4 * ci + h, :],
                                     start=(h == 0 and ci == 0),
                                     stop=(h == HPC - 1 and ci == NCORES - 1))
            ot = stg_pool.tile([128, HPC * D], F32, tag="ot")
            nc.scalar.copy(out=ot, in_=po)
            nc.gpsimd.dma_start(out=out_ext[qc * 128:(qc + 1) * 128, :], in_=ot)

    nc.compile()
    return nc


def _host_prep(context, query, w_qkv, w_o, q_norm_w, k_norm_w):
    context = np.asarray(context, dtype=np.float32)
    query = np.asarray(query, dtype=np.float32)
    w_qkv = np.asarray(w_qkv, dtype=np.float32)
    w_o = np.asarray(w_o, dtype=np.float32)
    q_norm_w = np.asarray(q_norm_w, dtype=np.float32)
    k_norm_w = np.asarray(k_norm_w, dtype=np.float32)

    ck = np.concatenate([context, query], axis=0)  # [KV, HID]
    ckT = np.ascontiguousarray(ck.T).astype(bfloat16)  # [HID, KV]

    wq = w_qkv[:, :H * D]
    wk = w_qkv[:, H * D:H * D + KVH * D]
    wv = w_qkv[:, H * D + KVH * D:]

    half = D // 2
    inv_freq = (1.0 / (THETA ** (np.arange(0, half, dtype=np.float32) / half))).astype(np.float32)
    pos = np.arange(KV, dtype=np.float32)
    freqs = pos[:, None] * inv_freq[None, :]
    cs = np.concatenate([np.cos(freqs), np.sin(freqs)], axis=1).astype(np.float32)  # [KV, D]

    nw = np.concatenate([
        np.broadcast_to(q_norm_w[None, :], (128, D)),
        np.broadcast_to(k_norm_w[None, :], (128, D)),
    ], axis=1).astype(np.float32)  # [128, 2D]

    p = np.arange(128)[:, None]
    q = np.arange(512)[None, :]
    msk = np.concatenate(
        [np.where(128 * i + p <= q, 0.0, MASKVAL) for i in range(4)],
        axis=1).astype(np.float32)  # [128, 2048]

    in_maps = []
    for c in range(NCORES):
        in_maps.append({
            "ckT": ckT,
            "wq": np.ascontiguousarray(wq[:, c * HPC * D:(c + 1) * HPC * D]).astype(bfloat16),
            "wkv": np.ascontiguousarray(
                np.concatenate([wk[:, c * D:(c + 1) * D], wv[:, c * D:(c + 1) * D]], axis=1)
            ).astype(bfloat16),
            "wo": np.ascontiguousarray(w_o[:, c * HPC * D:(c + 1) * HPC * D]).astype(bfloat16),
            "cs": cs,
            "nw": nw,
            "msk": msk,
        })
    return in_maps


def kernel(context, query, w_qkv, w_o, q_norm_w, k_norm_w, **kw):
    if "nc" not in _STATE:
        _STATE["nc"] = _build()
    nc = _STATE["nc"]
    in_maps = _host_prep(context, query, w_qkv, w_o, q_norm_w, k_norm_w)
    res = run_bass_kernel_spmd(nc, in_maps, list(range(NCORES)), **kw)
    out = np.concatenate([np.asarray(res.results[c]["out"]) for c in range(NCORES)], axis=1)
    if kw:
        return out.astype(np.float32), res
    return out.astype(np.float32)



# revision 9
# speedup vs baseline: 1.0365x; 1.0365x over previous
"""DFlash Qwen3 cross-attention on 8 TRN2 NeuronCores.

Sharding: tensor-parallel over heads. Core c owns KV head c (KVH=8) and the
4 query heads 4c..4c+3 of its GQA group.

Structure (v2, rebuilt from baseline trace analysis):
- All DMAs issued on the sync engine (HWDGE) -- the baseline's gpsimd
  (SWDGE) dma_starts cost ~1us of Q7 descriptor-gen each (455us total).
- Q/K norm+rope slimmed: scalar_tensor_tensor fusions with accum_out for
  the sum-of-squares, reciprocal_approx_fast, and the per-head RMSNorm
  weight folded into host-precomputed cos/sin tiles.  Q-norm runs on
  vector, K-norm on gpsimd so the two chains overlap the projection
  matmuls.  Norm batches are emitted one group behind the matmuls so the
  tensor queue never waits on the vector/gpsimd chains.
- Attention: j-tile outermost, head-pairs inner; S^T for both heads of a
  pair land in one 2-bank PSUM tile so ONE [128,1024] exp serves both
  (amortizes the ACT engine's 352-cycle fixed cost -- exp was the
  binding 722ns/pair in the baseline).  Softmax accumulation alternates
  between vector and gpsimd into two partial accumulators.
- o_proj for j-tile jq is emitted after the sweeps of jq+1, so each
  AllGather has a full j-sweep (~100us) to complete off the critical
  path; AGs are per (head-pair, j) of [128,1024]bf16.
"""

from contextlib import ExitStack

import numpy as np
from ml_dtypes import bfloat16

import concourse.bass as bass
import concourse.bass_isa as bass_isa
import concourse.mybir as mybir
import concourse.tile as tile
from concourse import bacc
from concourse.bass_utils import run_bass_kernel_spmd
from concourse.masks import make_identity

H = 32
KVH = 8
D = 128
HID = 4096
CTX = 4096
QL = 2048
KV = CTX + QL  # 6144
NCORES = 8
HPC = H // NCORES  # 4 q heads per core
THETA = 1000000.0
EPS = 1e-6
SCALE = float(D) ** -0.5

NHD = HID // 128  # 32 contraction chunks
NKV = KV // 128  # 48 kv chunks
NQC = QL // 128  # 16 q row chunks
NQJ = QL // 512  # 4 q column tiles for attention
MASKVAL = -1e6

F32 = mybir.dt.float32
BF16 = mybir.dt.bfloat16
MULT = mybir.AluOpType.mult

_STATE = {}


def _build():
    nc = bacc.Bacc()

    ckT = nc.declare_dram_parameter("ckT", [HID, KV], BF16, isOutput=False)
    wq = nc.declare_dram_parameter("wq", [HID, HPC * D], BF16, isOutput=False)
    wkv = nc.declare_dram_parameter("wkv", [HID, 2 * D], BF16, isOutput=False)
    wo = nc.declare_dram_parameter("wo", [HID, HPC * D], BF16, isOutput=False)
    csq = nc.declare_dram_parameter("csq", [QL, 2 * D], BF16, isOutput=False)
    csk = nc.declare_dram_parameter("csk", [KV, 2 * D], BF16, isOutput=False)
    msk = nc.declare_dram_parameter("msk", [128, 4 * 512], BF16, isOutput=False)
    out_ext = nc.declare_dram_parameter("out", [QL, HPC * D], F32, isOutput=True)

    # per-(head-pair, j-tile) AllGather buffers
    ag_ins = [[nc.dram_tensor(f"ag_in{hp}_{j}", [128, 1024], BF16) for j in range(NQJ)]
              for hp in range(2)]
    ag_outs = [[nc.dram_tensor(f"ag_out{hp}_{j}", [NCORES * 128, 1024], BF16,
                               addr_space="Shared") for j in range(NQJ)]
               for hp in range(2)]

    with tile.TileContext(nc) as tc, ExitStack() as ctx:
        singles = ctx.enter_context(tc.tile_pool(name="singles", bufs=1))
        ck_pool = ctx.enter_context(tc.tile_pool(name="ckp", bufs=4))
        wq_pool = ctx.enter_context(tc.tile_pool(name="wqp", bufs=3))
        cs_pool = ctx.enter_context(tc.tile_pool(name="csp", bufs=2))
        evac_pool = ctx.enter_context(tc.tile_pool(name="evac", bufs=3))
        tmp_pool = ctx.enter_context(tc.tile_pool(name="tmp", bufs=2))
        pt_pool = ctx.enter_context(tc.tile_pool(name="ptp", bufs=6))
        sacc_pool = ctx.enter_context(tc.tile_pool(name="sacc", bufs=1))
        stg_pool = ctx.enter_context(tc.tile_pool(name="stg", bufs=2))
        at_pool = ctx.enter_context(tc.tile_pool(name="atp", bufs=2))
        # PSUM: A = proj accumulators / attention S^T (2 x 2 banks);
        # B = o_acc (2 banks) + transpose staging (1 bank) + o_proj (1 bank)
        psumA = ctx.enter_context(tc.tile_pool(name="psA", bufs=2, space="PSUM"))
        psumB = ctx.enter_context(tc.tile_pool(name="psB", bufs=1, space="PSUM"))

        # ---- resident tensors ----
        wkv_sb = singles.tile([128, NHD, 2 * D], BF16)
        nc.sync.dma_start(out=wkv_sb[:], in_=wkv[:, :].rearrange("(k p) n -> p k n", p=128))
        wo_sb = singles.tile([128, NHD, HPC * D], BF16)
        nc.sync.dma_start(out=wo_sb[:], in_=wo[:, :].rearrange("(k p) n -> p k n", p=128))
        msk_sb = singles.tile([128, 4, 512], BF16)
        nc.sync.dma_start(out=msk_sb[:], in_=msk[:, :].rearrange("p (i c) -> p i c", i=4))

        identb = singles.tile([128, 128], BF16)
        make_identity(nc, identb)
        epst = singles.tile([128, 1], F32)
        nc.vector.memset(epst, EPS)
        zbias = singles.tile([128, 1], F32)
        nc.vector.memset(zbias, 0.0)

        qT_sb = singles.tile([128, HPC, QL], BF16)  # Q^T per head: [d, h, q]
        kT_sb = singles.tile([128, KV], BF16)  # K^T: [d, kv]
        v_sb = singles.tile([128, NKV, D], BF16)  # V: [kv%128, r, d]

        deferred = [None]

        def flush_deferred():
            if deferred[0] is not None:
                deferred[0]()
                deferred[0] = None

        # ================= Q projection =================
        # groups of 4 q row-chunks; norm+rope+transpose emitted one group
        # behind the matmuls so the tensor queue never waits on vector.
        def q_norm_batch(g, pq01):
            qe = evac_pool.tile([128, 4, 512], F32, tag="qe", bufs=2)
            nc.scalar.copy(out=qe[:, 0:2, :], in_=pq01[0])
            nc.scalar.copy(out=qe[:, 2:4, :], in_=pq01[1])
            cst = cs_pool.tile([128, 4, 2 * D], BF16, tag="csq")
            nc.sync.dma_start(out=cst, in_=csq[g * 512:(g + 1) * 512, :].rearrange(
                "(f p) c -> p f c", p=128))
            ssq = tmp_pool.tile([128, 16], F32, tag="ssq")
            sqs = tmp_pool.tile([128, 128], F32, tag="sqs")
            for qi in range(4):
                for h in range(HPC):
                    x = qe[:, qi, h * D:(h + 1) * D]
                    nc.vector.scalar_tensor_tensor(
                        out=sqs, in0=x, scalar=1.0, in1=x, op0=MULT, op1=MULT,
                        accum_out=ssq[:, qi * 4 + h: qi * 4 + h + 1])
            rms = tmp_pool.tile([128, 16], F32, tag="rms")
            nc.scalar.activation(out=rms, in_=ssq, func=mybir.ActivationFunctionType.Sqrt,
                                 bias=epst, scale=1.0 / D)
            rrms = tmp_pool.tile([128, 16], F32, tag="rrms")
            nc.vector.reciprocal_approx_fast(out=rrms, in_=rms)
            for qi in range(4):
                tp = psumB.tile([128, 512], BF16, tag="tp")
                for h in range(HPC):
                    x = qe[:, qi, h * D:(h + 1) * D]
                    rr = rrms[:, qi * 4 + h: qi * 4 + h + 1]
                    t1 = tmp_pool.tile([128, 128], F32, tag="t1")
                    nc.vector.scalar_tensor_tensor(
                        out=t1, in0=x, scalar=rr, in1=cst[:, qi, 0:128],
                        op0=MULT, op1=MULT)
                    t2 = tmp_pool.tile([128, 128], F32, tag="t2")
                    nc.vector.scalar_tensor_tensor(
                        out=t2[:, 0:64], in0=x[:, 64:128], scalar=rr,
                        in1=cst[:, qi, 128:192], op0=MULT, op1=MULT)
                    nc.vector.scalar_tensor_tensor(
                        out=t2[:, 64:128], in0=x[:, 0:64], scalar=rr,
                        in1=cst[:, qi, 192:256], op0=MULT, op1=MULT)
                    ro = tmp_pool.tile([128, 128], BF16, tag="ro")
                    nc.vector.tensor_add(ro, t1, t2)
                    nc.tensor.transpose(tp[:, h * 128:(h + 1) * 128], ro, identb)
                qc = g * 4 + qi
                nc.scalar.copy(out=qT_sb[:, :, qc * 128:(qc + 1) * 128],
                               in_=tp.rearrange("p (h q) -> p h q", h=4))

        for g in range(4):
            pq01 = [psumA.tile([128, 2, 512], F32, tag="acc", name=f"pq{g}_{i}")
                    for i in range(2)]
            for k2 in range(NHD // 2):
                ckq = ck_pool.tile([128, 2, 512], BF16, tag="ck")
                nc.sync.dma_start(
                    out=ckq,
                    in_=ckT[k2 * 256:(k2 + 1) * 256,
                            CTX + g * 512: CTX + (g + 1) * 512].rearrange(
                        "(two p) c -> p two c", p=128))
                wqt = wq_pool.tile([128, 2, 512], BF16, tag="wq")
                nc.sync.dma_start(
                    out=wqt,
                    in_=wq[k2 * 256:(k2 + 1) * 256, :].rearrange("(two p) c -> p two c", p=128))
                for two in range(2):
                    k = 2 * k2 + two
                    for qi in range(4):
                        nc.tensor.matmul(pq01[qi // 2][:, qi % 2, :],
                                         lhsT=ckq[:, two, (qi % 4) * 128:(qi % 4 + 1) * 128],
                                         rhs=wqt[:, two, :], start=(k == 0), stop=(k == NHD - 1))
            flush_deferred()
            deferred[0] = (lambda g=g, pq01=pq01: q_norm_batch(g, pq01))

        # ================= K/V projection =================
        def kv_norm_batch(rq, pk01):
            ke = evac_pool.tile([128, 4, 2 * D], F32, tag="ke", bufs=2)
            nc.scalar.copy(out=ke[:, 0:2, :], in_=pk01[0][:, :, 0:2 * D])
            nc.scalar.copy(out=ke[:, 2:4, :], in_=pk01[1][:, :, 0:2 * D])
            cst = cs_pool.tile([128, 4, 2 * D], BF16, tag="csk")
            nc.sync.dma_start(out=cst, in_=csk[rq * 512:(rq + 1) * 512, :].rearrange(
                "(f p) c -> p f c", p=128))
            ssqk = tmp_pool.tile([128, 4], F32, tag="ssqk")
            sqsk = tmp_pool.tile([128, 128], F32, tag="sqsk")
            for rr in range(4):
                x = ke[:, rr, 0:D]
                nc.vector.scalar_tensor_tensor(
                    out=sqsk, in0=x, scalar=1.0, in1=x, op0=MULT, op1=MULT,
                    accum_out=ssqk[:, rr:rr + 1])
            rmsk = tmp_pool.tile([128, 4], F32, tag="rmsk")
            nc.scalar.activation(out=rmsk, in_=ssqk, func=mybir.ActivationFunctionType.Sqrt,
                                 bias=epst, scale=1.0 / D)
            rrmsk = tmp_pool.tile([128, 4], F32, tag="rrmsk")
            nc.vector.reciprocal_approx_fast(out=rrmsk, in_=rmsk)
            tpk = psumB.tile([128, 512], BF16, tag="tp")
            for rr in range(4):
                x = ke[:, rr, 0:D]
                rr_s = rrmsk[:, rr:rr + 1]
                xn = tmp_pool.tile([128, 128], F32, tag="xn")
                nc.gpsimd.tensor_scalar_mul(out=xn, in0=x, scalar1=rr_s)
                t1k = tmp_pool.tile([128, 128], F32, tag="t1k")
                nc.gpsimd.tensor_mul(t1k, xn, cst[:, rr, 0:128])
                t2k = tmp_pool.tile([128, 128], F32, tag="t2k")
                nc.gpsimd.tensor_mul(t2k[:, 0:64], xn[:, 64:128], cst[:, rr, 128:192])
                nc.gpsimd.tensor_mul(t2k[:, 64:128], xn[:, 0:64], cst[:, rr, 192:256])
                rok = tmp_pool.tile([128, 128], BF16, tag="rok")
                nc.gpsimd.tensor_add(rok, t1k, t2k)
                nc.tensor.transpose(tpk[:, rr * 128:(rr + 1) * 128], rok, identb)
            nc.scalar.copy(out=kT_sb[:, rq * 512:(rq + 1) * 512], in_=tpk)
            # V: natural layout, cast to bf16 (from the SBUF evac)
            nc.gpsimd.tensor_copy(out=v_sb[:, rq * 4:rq * 4 + 4, :],
                                  in_=ke[:, :, D:2 * D])

        for rq in range(NKV // 4):
            pk01 = [psumA.tile([128, 2, 512], F32, tag="acc", name=f"pk{rq}_{i}")
                    for i in range(2)]
            for k2 in range(NHD // 2):
                ckt = ck_pool.tile([128, 2, 512], BF16, tag="ck")
                nc.sync.dma_start(
                    out=ckt,
                    in_=ckT[k2 * 256:(k2 + 1) * 256,
                            rq * 512:(rq + 1) * 512].rearrange("(two p) c -> p two c", p=128))
                for two in range(2):
                    k = 2 * k2 + two
                    for rr in range(4):
                        nc.tensor.matmul(pk01[rr // 2][:, rr % 2, 0:2 * D],
                                         lhsT=ckt[:, two, rr * 128:(rr + 1) * 128],
                                         rhs=wkv_sb[:, k, :], start=(k == 0), stop=(k == NHD - 1))
            flush_deferred()
            deferred[0] = (lambda rq=rq, pk01=pk01: kv_norm_batch(rq, pk01))

        flush_deferred()

        # ================= attention =================
        # j outermost (early AG -> o_proj overlap), head pairs share the
        # kT/v stationaries; one [128,1024] exp per r-step per pair.
        ones_rg = [list(range(NCORES))]

        def emit_oproj(jq):
            for qc in range(4 * jq, 4 * jq + 4):
                qo = (qc % 4) * 128
                po = psumB.tile([128, 512], F32, tag="po")
                first = True
                for hp in range(2):
                    for hl in range(2):
                        at = at_pool.tile([128, NCORES, 128], BF16, tag="at")
                        nc.sync.dma_start(
                            out=at,
                            in_=ag_outs[hp][jq][:, hl * 512 + qo: hl * 512 + qo + 128]
                            .rearrange("(c p) q -> p c q", p=128))
                        for ci in range(NCORES):
                            nc.tensor.matmul(po, lhsT=at[:, ci, :],
                                             rhs=wo_sb[:, HPC * ci + 2 * hp + hl, :],
                                             start=first,
                                             stop=(hp == 1 and hl == 1 and ci == NCORES - 1))
                            first = False
                ot = stg_pool.tile([128, 512], F32, tag="ot")
                nc.vector.tensor_copy(out=ot, in_=po)
                nc.sync.dma_start(out=out_ext[qc * 128:(qc + 1) * 128, :], in_=ot)

        for j in range(NQJ):
            rmax = 35 + 4 * j if j < NQJ - 1 else NKV - 1
            for hp in range(2):
                sacc_v = sacc_pool.tile([128, 2, 512], F32, tag="saccv", bufs=2)
                nc.vector.memset(sacc_v, 0.0)
                sacc_g = sacc_pool.tile([128, 2, 512], F32, tag="saccg", bufs=2)
                nc.gpsimd.memset(sacc_g, 0.0)
                o_acc = psumB.tile([128, 2, 512], F32, tag="oacc")
                for r in range(rmax + 1):
                    st = psumA.tile([128, 2, 512], F32, tag="acc")
                    for hl in range(2):
                        nc.tensor.matmul(st[:, hl, :], lhsT=kT_sb[:, r * 128:(r + 1) * 128],
                                         rhs=qT_sb[:, 2 * hp + hl, j * 512:(j + 1) * 512],
                                         start=True, stop=True)
                    i = r - 32 - 4 * j
                    if i >= 0:
                        w = 128 * (i + 1)
                        for hl in range(2):
                            nc.vector.tensor_add(st[:, hl, 0:w], st[:, hl, 0:w],
                                                 msk_sb[:, i, 0:w])
                    pt = pt_pool.tile([128, 2, 512], BF16, tag="pt")
                    nc.scalar.activation(out=pt, in_=st,
                                         func=mybir.ActivationFunctionType.Exp,
                                         bias=zbias, scale=SCALE)
                    if r % 2 == 0:
                        nc.vector.tensor_add(sacc_v, sacc_v, pt)
                    else:
                        nc.gpsimd.tensor_add(sacc_g, sacc_g, pt)
                    for hl in range(2):
                        nc.tensor.matmul(o_acc[:, hl, :], lhsT=v_sb[:, r, :],
                                         rhs=pt[:, hl, :],
                                         start=(r == 0), stop=(r == rmax))
                # sweep end: denominator, normalize, stage, AllGather
                den = sacc_pool.tile([128, 2, 512], F32, tag="den")
                nc.vector.tensor_add(den, sacc_v, sacc_g)
                pr = sacc_pool.tile([128, 2, 512], F32, tag="pr")
                nc.gpsimd.partition_all_reduce(pr, den, channels=128,
                                               reduce_op=bass_isa.ReduceOp.add)
                pri = sacc_pool.tile([128, 2, 512], F32, tag="pri")
                nc.vector.reciprocal_approx_fast(out=pri, in_=pr)
                o_raw = sacc_pool.tile([128, 2, 512], F32, tag="oraw")
                nc.scalar.copy(out=o_raw, in_=o_acc)
                stg = stg_pool.tile([128, 2, 512], BF16, tag="stg")
                nc.vector.tensor_mul(stg, o_raw, pri)
                nc.sync.dma_start(out=ag_ins[hp][j][:], in_=stg)
                nc.gpsimd.collective_compute(
                    "AllGather",
                    mybir.AluOpType.bypass,
                    ins=[ag_ins[hp][j][:]],
                    outs=[ag_outs[hp][j][:]],
                    replica_groups=ones_rg,
                )
            if j >= 1:
                emit_oproj(j - 1)
        emit_oproj(NQJ - 1)

    nc.compile()
    return nc


def _host_prep(context, query, w_qkv, w_o, q_norm_w, k_norm_w):
    context = np.asarray(context, dtype=np.float32)
    query = np.asarray(query, dtype=np.float32)
    w_qkv = np.asarray(w_qkv, dtype=np.float32)
    w_o = np.asarray(w_o, dtype=np.float32)
    q_norm_w = np.asarray(q_norm_w, dtype=np.float32)
    k_norm_w = np.asarray(k_norm_w, dtype=np.float32)

    ck = np.concatenate([context, query], axis=0)  # [KV, HID]
    ckT = np.ascontiguousarray(ck.T).astype(bfloat16)  # [HID, KV]

    wq = w_qkv[:, :H * D]
    wk = w_qkv[:, H * D:H * D + KVH * D]
    wv = w_qkv[:, H * D + KVH * D:]

    half = D // 2
    inv_freq = (1.0 / (THETA ** (np.arange(0, half, dtype=np.float32) / half))).astype(np.float32)
    pos = np.arange(KV, dtype=np.float32)
    freqs = pos[:, None] * inv_freq[None, :]
    c = np.cos(freqs)
    s = np.sin(freqs)

    def make_cs(nw):
        nw1 = nw[None, :half]
        nw2 = nw[None, half:]
        A = np.concatenate([c * nw1, c * nw2], axis=1)       # [KV, 128]
        B = np.concatenate([-s * nw2, s * nw1], axis=1)      # [KV, 128]
        return np.concatenate([A, B], axis=1).astype(bfloat16)  # [KV, 256]

    csk_full = make_cs(k_norm_w)
    csq_full = make_cs(q_norm_w)[CTX:]

    p = np.arange(128)[:, None]
    q = np.arange(512)[None, :]
    msk = np.concatenate(
        [np.where(128 * i + p <= q, 0.0, MASKVAL) for i in range(4)],
        axis=1).astype(bfloat16)  # [128, 2048]

    in_maps = []
    for cidx in range(NCORES):
        in_maps.append({
            "ckT": ckT,
            "wq": np.ascontiguousarray(wq[:, cidx * HPC * D:(cidx + 1) * HPC * D]).astype(bfloat16),
            "wkv": np.ascontiguousarray(
                np.concatenate([wk[:, cidx * D:(cidx + 1) * D], wv[:, cidx * D:(cidx + 1) * D]], axis=1)
            ).astype(bfloat16),
            "wo": np.ascontiguousarray(w_o[:, cidx * HPC * D:(cidx + 1) * HPC * D]).astype(bfloat16),
            "csq": csq_full,
            "csk": csk_full,
            "msk": msk,
        })
    return in_maps


def kernel(context, query, w_qkv, w_o, q_norm_w, k_norm_w, **kw):
    if "nc" not in _STATE:
        _STATE["nc"] = _build()
    nc = _STATE["nc"]
    in_maps = _host_prep(context, query, w_qkv, w_o, q_norm_w, k_norm_w)
    res = run_bass_kernel_spmd(nc, in_maps, list(range(NCORES)), **kw)
    out = np.concatenate([np.asarray(res.results[c]["out"]) for c in range(NCORES)], axis=1)
    if kw:
        return out.astype(np.float32), res
    return out.astype(np.float32)


# revision 13
# speedup vs baseline: 1.2111x; 1.1684x over previous
"""DFlash Qwen3 cross-attention on 8 TRN2 NeuronCores.

Sharding: tensor-parallel over heads. Core c owns KV head c (KVH=8) and the
4 query heads 4c..4c+3 of its GQA group.

v3 structure (from trace analysis of baseline + v2):
- All DMAs on HWDGE: big streams on the sync ring, small latency-sensitive
  loads (cos/sin) + AG staging + output stores on the scalar ring.
- Projections: matmul groups emitted ahead; the norm/rope math one group
  behind and the PE transposes two groups behind, so the tensor queue
  never waits on the vector/gpsimd norm chains.
- Q norm+rope on vector (scalar_tensor_tensor fusions, rms weight folded
  into host cos/sin tiles, reciprocal_approx_fast); K norm split: squares
  + scalar-mul on vector (Pool's tensor_scalar is a slow Q7 ucode), the
  rope tensor_tensor multiplies on gpsimd (native, fast).
- Attention: j outermost (AG -> o_proj one j behind), head pairs share
  kT/v stationaries, ONE [128,1024] exp per r-step, softmax denominator
  accumulated on vector only via bf16 pair-sums (pt_r + pt_{r+1} in bf16
  at 2 elem/cycle, then one f32 accumulate per pair -- ~1.02us/step).
  Sweep end: partition_all_reduce on the (otherwise idle) gpsimd,
  reciprocal_approx_fast, normalize straight out of PSUM.
"""

from contextlib import ExitStack

import numpy as np
from ml_dtypes import bfloat16

import concourse.bass as bass
import concourse.bass_isa as bass_isa
import concourse.mybir as mybir
import concourse.tile as tile
from concourse import bacc
from concourse.bass_utils import run_bass_kernel_spmd
from concourse.masks import make_identity

H = 32
KVH = 8
D = 128
HID = 4096
CTX = 4096
QL = 2048
KV = CTX + QL  # 6144
NCORES = 8
HPC = H // NCORES  # 4 q heads per core
THETA = 1000000.0
EPS = 1e-6
SCALE = float(D) ** -0.5

NHD = HID // 128  # 32 contraction chunks
NKV = KV // 128  # 48 kv chunks
NQC = QL // 128  # 16 q row chunks
NQJ = QL // 512  # 4 q column tiles for attention
MASKVAL = -1e6

F32 = mybir.dt.float32
BF16 = mybir.dt.bfloat16
MULT = mybir.AluOpType.mult

_STATE = {}


def _build():
    nc = bacc.Bacc()

    ckT = nc.declare_dram_parameter("ckT", [HID, KV], BF16, isOutput=False)
    wq = nc.declare_dram_parameter("wq", [HID, HPC * D], BF16, isOutput=False)
    wkv = nc.declare_dram_parameter("wkv", [HID, 2 * D], BF16, isOutput=False)
    wo = nc.declare_dram_parameter("wo", [HID, HPC * D], BF16, isOutput=False)
    csq = nc.declare_dram_parameter("csq", [QL, 2 * D], BF16, isOutput=False)
    csk = nc.declare_dram_parameter("csk", [KV, 2 * D], BF16, isOutput=False)
    msk = nc.declare_dram_parameter("msk", [128, 4 * 512], BF16, isOutput=False)
    out_ext = nc.declare_dram_parameter("out", [QL, HPC * D], F32, isOutput=True)

    ag_ins = [[nc.dram_tensor(f"ag_in{hp}_{j}", [128, 1024], BF16) for j in range(NQJ)]
              for hp in range(2)]
    ag_outs = [[nc.dram_tensor(f"ag_out{hp}_{j}", [NCORES * 128, 1024], BF16,
                               addr_space="Shared") for j in range(NQJ)]
               for hp in range(2)]

    with tile.TileContext(nc) as tc, ExitStack() as ctx:
        singles = ctx.enter_context(tc.tile_pool(name="singles", bufs=1))
        ck_pool = ctx.enter_context(tc.tile_pool(name="ckp", bufs=4))
        wq_pool = ctx.enter_context(tc.tile_pool(name="wqp", bufs=3))
        cs_pool = ctx.enter_context(tc.tile_pool(name="csp", bufs=3))
        evac_pool = ctx.enter_context(tc.tile_pool(name="evac", bufs=3))
        tmp_pool = ctx.enter_context(tc.tile_pool(name="tmp", bufs=3))
        pt_pool = ctx.enter_context(tc.tile_pool(name="ptp", bufs=6))
        sacc_pool = ctx.enter_context(tc.tile_pool(name="sacc", bufs=1))
        stg_pool = ctx.enter_context(tc.tile_pool(name="stg", bufs=2))
        at_pool = ctx.enter_context(tc.tile_pool(name="atp", bufs=2))
        # PSUM: A = proj accumulators / attention S^T (2 x 2 banks = 4);
        # B: "oacc" (2 banks) + "scr" (2 banks, shared: proj transposes,
        # o_proj accumulator)
        psumA = ctx.enter_context(tc.tile_pool(name="psA", bufs=2, space="PSUM"))
        psumB = ctx.enter_context(tc.tile_pool(name="psB", bufs=1, space="PSUM"))

        # ---- resident tensors ----
        wkv_sb = singles.tile([128, NHD, 2 * D], BF16)
        nc.sync.dma_start(out=wkv_sb[:], in_=wkv[:, :].rearrange("(k p) n -> p k n", p=128))
        wo_sb = singles.tile([128, NHD, HPC * D], BF16)
        nc.sync.dma_start(out=wo_sb[:], in_=wo[:, :].rearrange("(k p) n -> p k n", p=128))
        msk_sb = singles.tile([128, 4, 512], BF16)
        nc.scalar.dma_start(out=msk_sb[:], in_=msk[:, :].rearrange("p (i c) -> p i c", i=4))

        identb = singles.tile([128, 128], BF16)
        make_identity(nc, identb)
        epst = singles.tile([128, 1], F32)
        nc.vector.memset(epst, EPS)
        zbias = singles.tile([128, 1], F32)
        nc.vector.memset(zbias, 0.0)

        qT_sb = singles.tile([128, HPC, QL], BF16)  # Q^T per head: [d, h, q]
        kT_sb = singles.tile([128, KV], BF16)  # K^T: [d, kv]
        v_sb = singles.tile([128, NKV, D], BF16)  # V: [kv%128, r, d]

        # deferred emission: stage1 (evac + norm math) runs one MM-group
        # behind, stage2 (PE transposes + copies) two groups behind.
        pend1 = [None]
        pend2 = [None]

        def step_pipeline(new_stage1=None):
            if pend2[0] is not None:
                pend2[0]()
            pend2[0] = None
            if pend1[0] is not None:
                pend2[0] = pend1[0]()
            pend1[0] = new_stage1

        def scr_tile(shape, dtype, parity):
            tag = "scr" if parity % 2 == 0 else "oacc"
            return psumB.tile(shape, dtype, tag=tag, name=f"scr_{tag}")

        # ================= Q projection =================
        def q_stage1(g, pq01):
            qe = evac_pool.tile([128, 4, 512], F32, tag="qe", bufs=2)
            nc.scalar.copy(out=qe[:, 0:2, :], in_=pq01[0])
            nc.scalar.copy(out=qe[:, 2:4, :], in_=pq01[1])
            cst = cs_pool.tile([128, 4, 2 * D], BF16, tag="cs")
            nc.scalar.dma_start(out=cst, in_=csq[g * 512:(g + 1) * 512, :].rearrange(
                "(f p) c -> p f c", p=128))
            ssq = tmp_pool.tile([128, 16], F32, tag="ssq")
            sqs = tmp_pool.tile([128, 128], F32, tag="sqs")
            for qi in range(4):
                for h in range(HPC):
                    x = qe[:, qi, h * D:(h + 1) * D]
                    nc.vector.scalar_tensor_tensor(
                        out=sqs, in0=x, scalar=1.0, in1=x, op0=MULT, op1=MULT,
                        accum_out=ssq[:, qi * 4 + h: qi * 4 + h + 1])
            rms = tmp_pool.tile([128, 16], F32, tag="rms")
            nc.scalar.activation(out=rms, in_=ssq, func=mybir.ActivationFunctionType.Sqrt,
                                 bias=epst, scale=1.0 / D)
            rrms = tmp_pool.tile([128, 16], F32, tag="rrms")
            nc.vector.reciprocal_approx_fast(out=rrms, in_=rms)
            ros = []
            for qi in range(4):
                for h in range(HPC):
                    x = qe[:, qi, h * D:(h + 1) * D]
                    rr = rrms[:, qi * 4 + h: qi * 4 + h + 1]
                    t1 = tmp_pool.tile([128, 128], F32, tag="t1")
                    nc.vector.scalar_tensor_tensor(
                        out=t1, in0=x, scalar=rr, in1=cst[:, qi, 0:128],
                        op0=MULT, op1=MULT)
                    t2 = tmp_pool.tile([128, 128], F32, tag="t2")
                    nc.vector.scalar_tensor_tensor(
                        out=t2[:, 0:64], in0=x[:, 64:128], scalar=rr,
                        in1=cst[:, qi, 128:192], op0=MULT, op1=MULT)
                    nc.vector.scalar_tensor_tensor(
                        out=t2[:, 64:128], in0=x[:, 0:64], scalar=rr,
                        in1=cst[:, qi, 192:256], op0=MULT, op1=MULT)
                    ro = tmp_pool.tile([128, 128], BF16, tag="ro", bufs=33)
                    nc.vector.tensor_add(ro, t1, t2)
                    ros.append(ro)

            def q_stage2(g=g, ros=ros):
                for qi in range(4):
                    tp = scr_tile([128, 512], BF16, qi)
                    for h in range(HPC):
                        nc.tensor.transpose(tp[:, h * 128:(h + 1) * 128],
                                            ros[qi * 4 + h], identb)
                    qc = g * 4 + qi
                    nc.scalar.copy(out=qT_sb[:, :, qc * 128:(qc + 1) * 128],
                                   in_=tp.rearrange("p (h q) -> p h q", h=4))
            return q_stage2

        for g in range(4):
            pq01 = [psumA.tile([128, 2, 512], F32, tag="acc", name=f"pq{g}_{i}")
                    for i in range(2)]
            for k2 in range(NHD // 2):
                ckq = ck_pool.tile([128, 2, 512], BF16, tag="ck")
                nc.sync.dma_start(
                    out=ckq,
                    in_=ckT[k2 * 256:(k2 + 1) * 256,
                            CTX + g * 512: CTX + (g + 1) * 512].rearrange(
                        "(two p) c -> p two c", p=128))
                wqt = wq_pool.tile([128, 2, 512], BF16, tag="wq")
                nc.sync.dma_start(
                    out=wqt,
                    in_=wq[k2 * 256:(k2 + 1) * 256, :].rearrange("(two p) c -> p two c", p=128))
                for two in range(2):
                    k = 2 * k2 + two
                    for qi in range(4):
                        nc.tensor.matmul(pq01[qi // 2][:, qi % 2, :],
                                         lhsT=ckq[:, two, qi * 128:(qi + 1) * 128],
                                         rhs=wqt[:, two, :], start=(k == 0), stop=(k == NHD - 1))
            step_pipeline(lambda g=g, pq01=pq01: q_stage1(g, pq01))

        # ================= K/V projection =================
        def kv_stage1(rq, pk01):
            ke = evac_pool.tile([128, 4, 2 * D], F32, tag="ke", bufs=3)
            nc.scalar.copy(out=ke[:, 0:2, :], in_=pk01[0][:, :, 0:2 * D])
            nc.scalar.copy(out=ke[:, 2:4, :], in_=pk01[1][:, :, 0:2 * D])
            cst = cs_pool.tile([128, 4, 2 * D], BF16, tag="cs")
            nc.scalar.dma_start(out=cst, in_=csk[rq * 512:(rq + 1) * 512, :].rearrange(
                "(f p) c -> p f c", p=128))
            ssqk = tmp_pool.tile([128, 4], F32, tag="ssqk")
            sqsk = tmp_pool.tile([128, 128], F32, tag="sqsk")
            for rr in range(4):
                x = ke[:, rr, 0:D]
                nc.vector.scalar_tensor_tensor(
                    out=sqsk, in0=x, scalar=1.0, in1=x, op0=MULT, op1=MULT,
                    accum_out=ssqk[:, rr:rr + 1])
            rmsk = tmp_pool.tile([128, 4], F32, tag="rmsk")
            nc.scalar.activation(out=rmsk, in_=ssqk, func=mybir.ActivationFunctionType.Sqrt,
                                 bias=epst, scale=1.0 / D)
            rrmsk = tmp_pool.tile([128, 4], F32, tag="rrmsk")
            nc.vector.reciprocal_approx_fast(out=rrmsk, in_=rmsk)
            roks = []
            for rr in range(4):
                x = ke[:, rr, 0:D]
                xn = tmp_pool.tile([128, 128], F32, tag="xn")
                nc.vector.tensor_scalar_mul(out=xn, in0=x, scalar1=rrmsk[:, rr:rr + 1])
                t1k = tmp_pool.tile([128, 128], F32, tag="t1k")
                nc.gpsimd.tensor_mul(t1k, xn, cst[:, rr, 0:128])
                t2k = tmp_pool.tile([128, 128], F32, tag="t2k")
                nc.gpsimd.tensor_mul(t2k[:, 0:64], xn[:, 64:128], cst[:, rr, 128:192])
                nc.gpsimd.tensor_mul(t2k[:, 64:128], xn[:, 0:64], cst[:, rr, 192:256])
                rok = tmp_pool.tile([128, 128], BF16, tag="rok", bufs=9)
                nc.gpsimd.tensor_add(rok, t1k, t2k)
                roks.append(rok)
            # V: natural layout, cast to bf16 (vector; Pool casts are slow)
            nc.vector.tensor_copy(out=v_sb[:, rq * 4:rq * 4 + 4, :],
                                  in_=ke[:, :, D:2 * D])

            def kv_stage2(rq=rq, roks=roks):
                tpk = scr_tile([128, 512], BF16, rq)
                for rr in range(4):
                    nc.tensor.transpose(tpk[:, rr * 128:(rr + 1) * 128], roks[rr], identb)
                nc.scalar.copy(out=kT_sb[:, rq * 512:(rq + 1) * 512], in_=tpk)
            return kv_stage2

        for rq in range(NKV // 4):
            pk01 = [psumA.tile([128, 2, 512], F32, tag="acc", name=f"pk{rq}_{i}")
                    for i in range(2)]
            for k2 in range(NHD // 2):
                ckt = ck_pool.tile([128, 2, 512], BF16, tag="ck")
                nc.sync.dma_start(
                    out=ckt,
                    in_=ckT[k2 * 256:(k2 + 1) * 256,
                            rq * 512:(rq + 1) * 512].rearrange("(two p) c -> p two c", p=128))
                for two in range(2):
                    k = 2 * k2 + two
                    for rr in range(4):
                        nc.tensor.matmul(pk01[rr // 2][:, rr % 2, 0:2 * D],
                                         lhsT=ckt[:, two, rr * 128:(rr + 1) * 128],
                                         rhs=wkv_sb[:, k, :], start=(k == 0), stop=(k == NHD - 1))
            step_pipeline(lambda rq=rq, pk01=pk01: kv_stage1(rq, pk01))

        step_pipeline()
        step_pipeline()

        # ================= attention =================
        ones_rg = [list(range(NCORES))]

        def emit_oproj(jq):
            for qc in range(4 * jq, 4 * jq + 4):
                qo = (qc % 4) * 128
                po = psumB.tile([128, 512], F32, tag="scr")
                first = True
                for hp in range(2):
                    for hl in range(2):
                        at = at_pool.tile([128, NCORES, 128], BF16, tag="at")
                        nc.sync.dma_start(
                            out=at,
                            in_=ag_outs[hp][jq][:, hl * 512 + qo: hl * 512 + qo + 128]
                            .rearrange("(c p) q -> p c q", p=128))
                        for ci in range(NCORES):
                            nc.tensor.matmul(po, lhsT=at[:, ci, :],
                                             rhs=wo_sb[:, HPC * ci + 2 * hp + hl, :],
                                             start=first,
                                             stop=(hp == 1 and hl == 1 and ci == NCORES - 1))
                            first = False
                ot = stg_pool.tile([128, 512], F32, tag="ot")
                nc.vector.tensor_copy(out=ot, in_=po)
                nc.scalar.dma_start(out=out_ext[qc * 128:(qc + 1) * 128, :], in_=ot)

        for j in range(NQJ):
            rmax = 35 + 4 * j if j < NQJ - 1 else NKV - 1
            for hp in range(2):
                sacc = sacc_pool.tile([128, 2, 512], F32, tag="sacc", bufs=2)
                nc.vector.memset(sacc, 0.0)
                o_acc = psumB.tile([128, 2, 512], F32, tag="oacc")
                pt_pair = []
                for r in range(rmax + 1):
                    st = psumA.tile([128, 2, 512], F32, tag="acc")
                    for hl in range(2):
                        nc.tensor.matmul(st[:, hl, :], lhsT=kT_sb[:, r * 128:(r + 1) * 128],
                                         rhs=qT_sb[:, 2 * hp + hl, j * 512:(j + 1) * 512],
                                         start=True, stop=True)
                    i = r - 32 - 4 * j
                    if i >= 0:
                        w = 128 * (i + 1)
                        for hl in range(2):
                            nc.vector.tensor_add(st[:, hl, 0:w], st[:, hl, 0:w],
                                                 msk_sb[:, i, 0:w])
                    pt = pt_pool.tile([128, 2, 512], BF16, tag="pt")
                    nc.scalar.activation(out=pt, in_=st,
                                         func=mybir.ActivationFunctionType.Exp,
                                         bias=zbias, scale=SCALE)
                    for hl in range(2):
                        nc.tensor.matmul(o_acc[:, hl, :], lhsT=v_sb[:, r, :],
                                         rhs=pt[:, hl, :],
                                         start=(r == 0), stop=(r == rmax))
                    # denominator: bf16 pair-sum (2 elem/cyc) + f32 accumulate
                    pt_pair.append(pt)
                    if len(pt_pair) == 2:
                        u = tmp_pool.tile([128, 2, 512], BF16, tag="u", bufs=2)
                        nc.vector.tensor_add(u, pt_pair[0], pt_pair[1])
                        nc.vector.tensor_add(sacc, sacc, u)
                        pt_pair = []
                if pt_pair:
                    nc.vector.tensor_add(sacc, sacc, pt_pair[0])
                    pt_pair = []
                # sweep end: partition-reduce (gpsimd), recip, normalize
                pr = sacc_pool.tile([128, 2, 512], F32, tag="pr")
                nc.gpsimd.partition_all_reduce(pr, sacc, channels=128,
                                               reduce_op=bass_isa.ReduceOp.add)
                pri = sacc_pool.tile([128, 2, 512], F32, tag="pri")
                nc.vector.reciprocal_approx_fast(out=pri, in_=pr)
                stg = stg_pool.tile([128, 2, 512], BF16, tag="stg")
                nc.vector.tensor_mul(stg, o_acc, pri)
                nc.scalar.dma_start(out=ag_ins[hp][j][:], in_=stg.rearrange("p a b -> p (a b)"))
                nc.gpsimd.collective_compute(
                    "AllGather",
                    mybir.AluOpType.bypass,
                    ins=[ag_ins[hp][j][:]],
                    outs=[ag_outs[hp][j][:]],
                    replica_groups=ones_rg,
                )
            if j >= 1:
                emit_oproj(j - 1)
        emit_oproj(NQJ - 1)

    nc.compile()
    return nc


def _host_prep(context, query, w_qkv, w_o, q_norm_w, k_norm_w):
    context = np.asarray(context, dtype=np.float32)
    query = np.asarray(query, dtype=np.float32)
    w_qkv = np.asarray(w_qkv, dtype=np.float32)
    w_o = np.asarray(w_o, dtype=np.float32)
    q_norm_w = np.asarray(q_norm_w, dtype=np.float32)
    k_norm_w = np.asarray(k_norm_w, dtype=np.float32)

    ck = np.concatenate([context, query], axis=0)  # [KV, HID]
    ckT = np.ascontiguousarray(ck.T).astype(bfloat16)  # [HID, KV]

    wq = w_qkv[:, :H * D]
    wk = w_qkv[:, H * D:H * D + KVH * D]
    wv = w_qkv[:, H * D + KVH * D:]

    half = D // 2
    inv_freq = (1.0 / (THETA ** (np.arange(0, half, dtype=np.float32) / half))).astype(np.float32)
    pos = np.arange(KV, dtype=np.float32)
    freqs = pos[:, None] * inv_freq[None, :]
    c = np.cos(freqs)
    s = np.sin(freqs)

    def make_cs(nw):
        nw1 = nw[None, :half]
        nw2 = nw[None, half:]
        A = np.concatenate([c * nw1, c * nw2], axis=1)       # [KV, 128]
        B = np.concatenate([-s * nw2, s * nw1], axis=1)      # [KV, 128]
        return np.concatenate([A, B], axis=1).astype(bfloat16)  # [KV, 256]

    csk_full = make_cs(k_norm_w)
    csq_full = make_cs(q_norm_w)[CTX:]

    p = np.arange(128)[:, None]
    q = np.arange(512)[None, :]
    msk = np.concatenate(
        [np.where(128 * i + p <= q, 0.0, MASKVAL) for i in range(4)],
        axis=1).astype(bfloat16)  # [128, 2048]

    in_maps = []
    for cidx in range(NCORES):
        in_maps.append({
            "ckT": ckT,
            "wq": np.ascontiguousarray(wq[:, cidx * HPC * D:(cidx + 1) * HPC * D]).astype(bfloat16),
            "wkv": np.ascontiguousarray(
                np.concatenate([wk[:, cidx * D:(cidx + 1) * D], wv[:, cidx * D:(cidx + 1) * D]], axis=1)
            ).astype(bfloat16),
            "wo": np.ascontiguousarray(w_o[:, cidx * HPC * D:(cidx + 1) * HPC * D]).astype(bfloat16),
            "csq": csq_full,
            "csk": csk_full,
            "msk": msk,
        })
    return in_maps


def kernel(context, query, w_qkv, w_o, q_norm_w, k_norm_w, **kw):
    if "nc" not in _STATE:
        _STATE["nc"] = _build()
    nc = _STATE["nc"]
    in_maps = _host_prep(context, query, w_qkv, w_o, q_norm_w, k_norm_w)
    res = run_bass_kernel_spmd(nc, in_maps, list(range(NCORES)), **kw)
    out = np.concatenate([np.asarray(res.results[c]["out"]) for c in range(NCORES)], axis=1)
    if kw:
        return out.astype(np.float32), res
    return out.astype(np.float32)


# revision 21
# speedup vs baseline: 1.2575x; 1.0383x over previous
"""DFlash Qwen3 cross-attention on 8 TRN2 NeuronCores.

Sharding: tensor-parallel over heads. Core c owns KV head c (KVH=8) and the
4 query heads 4c..4c+3 of its GQA group.

v4 structure (evolved from baseline/v2/v3 trace analysis):
- All DMAs on HWDGE rings (sync = big streams + o_proj loads, scalar =
  cos/sin loads, AG staging, output stores).  No SWDGE descriptor-gen.
- Both projections are W-stationary: lhsT = weight chunk, moving = ckT
  columns at N=512, so Q^T and K^T come out of PSUM directly in the
  [d, pos] layout attention wants (no per-chunk PE transposes) and the
  KV matmul count halves vs the ck-stationary form.  V^T is transposed
  back to natural via 48 PE transposes.
- RMSNorm in transposed space: evac PSUM->bf16 SBUF, square (DVE 2x),
  partition-sum via a bf16 ones-matmul on the tensor engine (output is
  broadcast over partitions), ACT sqrt + reciprocal_approx_fast, rope
  via two host-precomputed transposed cos/sin tiles (norm weight folded
  in), final per-column 1/rms multiply writes qT/kT directly.
- Attention: j outermost (AG -> o_proj one j-tile behind), head pairs
  share kT/v stationaries, ONE [128,1024] exp per r-step, softmax
  denominator on vector only via bf16 pair-sums + f32 accumulate.
- Sweep end: partition-reduce of the denominator via an f32 ones-matmul
  (tensor), reciprocal_approx_fast, normalize straight out of PSUM.
  gpsimd runs ONLY the collective triggers: the AllGather trigger blocks
  its queue until the collective completes (~20us), which in v3 starved
  partition_all_reduce and stalled the whole pipeline at j boundaries.
"""

from contextlib import ExitStack

import numpy as np
from ml_dtypes import bfloat16

import concourse.bass as bass
import concourse.bass_isa as bass_isa
import concourse.mybir as mybir
import concourse.tile as tile
from concourse import bacc
from concourse.bass_utils import run_bass_kernel_spmd
from concourse.masks import make_identity

H = 32
KVH = 8
D = 128
HID = 4096
CTX = 4096
QL = 2048
KV = CTX + QL  # 6144
NCORES = 8
HPC = H // NCORES  # 4 q heads per core
THETA = 1000000.0
EPS = 1e-6
SCALE = float(D) ** -0.5

NHD = HID // 128  # 32 contraction chunks
NKV = KV // 128  # 48 kv chunks
NQC = QL // 128  # 16 q row chunks
NQJ = QL // 512  # 4 q column tiles for attention
MASKVAL = -1e6

F32 = mybir.dt.float32
BF16 = mybir.dt.bfloat16
MULT = mybir.AluOpType.mult

_STATE = {}


def _build():
    nc = bacc.Bacc()

    ckT = nc.declare_dram_parameter("ckT", [HID, KV], BF16, isOutput=False)
    wq = nc.declare_dram_parameter("wq", [HID, HPC * D], BF16, isOutput=False)
    wkv = nc.declare_dram_parameter("wkv", [HID, 2 * D], BF16, isOutput=False)
    wo = nc.declare_dram_parameter("wo", [HID, HPC * D], BF16, isOutput=False)
    csqT = nc.declare_dram_parameter("csqT", [2 * D, QL], BF16, isOutput=False)
    cskT = nc.declare_dram_parameter("cskT", [2 * D, KV], BF16, isOutput=False)
    msk = nc.declare_dram_parameter("msk", [128, 4 * 512], BF16, isOutput=False)
    rot = nc.declare_dram_parameter("rot", [128, 128], BF16, isOutput=False)
    out_ext = nc.declare_dram_parameter("out", [QL, HPC * D], F32, isOutput=True)

    ag_ins = [[nc.dram_tensor(f"ag_in{hp}_{j}", [128, 1024], BF16) for j in range(NQJ)]
              for hp in range(2)]
    ag_outs = [[nc.dram_tensor(f"ag_out{hp}_{j}", [NCORES * 128, 1024], BF16,
                               addr_space="Shared") for j in range(NQJ)]
               for hp in range(2)]

    with tile.TileContext(nc) as tc, ExitStack() as ctx:
        singles = ctx.enter_context(tc.tile_pool(name="singles", bufs=1))
        ck_pool = ctx.enter_context(tc.tile_pool(name="ckp", bufs=4))
        cs_pool = ctx.enter_context(tc.tile_pool(name="csp", bufs=3))
        evac_pool = ctx.enter_context(tc.tile_pool(name="evac", bufs=3))
        tmp_pool = ctx.enter_context(tc.tile_pool(name="tmp", bufs=2))
        pt_pool = ctx.enter_context(tc.tile_pool(name="ptp", bufs=5))
        sacc_pool = ctx.enter_context(tc.tile_pool(name="sacc", bufs=1))
        stg_pool = ctx.enter_context(tc.tile_pool(name="stg", bufs=2))
        at_pool = ctx.enter_context(tc.tile_pool(name="atp", bufs=2))
        # PSUM: A = proj accumulators / attention S^T (2 x 2 banks = 4);
        # B: "oacc" (2 banks) + "scr" (2 banks: proj norm-sums + V
        # transposes, attention denominator, o_proj accumulator)
        psumA = ctx.enter_context(tc.tile_pool(name="psA", bufs=2, space="PSUM"))
        psumB = ctx.enter_context(tc.tile_pool(name="psB", bufs=1, space="PSUM"))

        # ---- resident tensors ----
        wq_sb = singles.tile([128, NHD, HPC * D], BF16)
        nc.sync.dma_start(out=wq_sb[:], in_=wq[:, :].rearrange("(k p) n -> p k n", p=128))
        wkv_sb = singles.tile([128, NHD, 2 * D], BF16)
        nc.sync.dma_start(out=wkv_sb[:], in_=wkv[:, :].rearrange("(k p) n -> p k n", p=128))
        wo_sb = singles.tile([128, NHD, HPC * D], BF16)
        nc.sync.dma_start(out=wo_sb[:], in_=wo[:, :].rearrange("(k p) n -> p k n", p=128))
        msk_sb = singles.tile([128, 4, 512], BF16)
        nc.scalar.dma_start(out=msk_sb[:], in_=msk[:, :].rearrange("p (i c) -> p i c", i=4))

        identb = singles.tile([128, 128], BF16)
        make_identity(nc, identb)
        rot_sb = singles.tile([128, 128], BF16)
        nc.scalar.dma_start(out=rot_sb, in_=rot[:, :])
        ones_b = singles.tile([128, 128], BF16)
        nc.vector.memset(ones_b, 1.0)
        ones_f = singles.tile([128, 128], F32)
        nc.vector.memset(ones_f, 1.0)
        epst = singles.tile([128, 1], F32)
        nc.vector.memset(epst, EPS)
        zbias = singles.tile([128, 1], F32)
        nc.vector.memset(zbias, 0.0)

        qT_sb = singles.tile([128, HPC, QL], BF16)  # Q^T per head: [d, h, q]
        kT_sb = singles.tile([128, KV], BF16)  # K^T: [d, kv]
        v_sb = singles.tile([128, NKV, D], BF16)  # V: [kv%128, r, d]

        pend1 = [None]
        pend2 = [None]

        def step_pipeline(new_stage1=None):
            if pend2[0] is not None:
                pend2[0]()
            pend2[0] = None
            if pend1[0] is not None:
                pend2[0] = pend1[0]()
            pend1[0] = new_stage1

        def norm_rope_T(xb, cst, out_slice):
            """Transposed-space rmsnorm+rope for one [128, 512] tile.

            xb: [128, 512] bf16 (rows = d, cols = positions).
            cst: [128, 2, 512] bf16 (A_T, B_T with norm weight folded in).
            The d -> (d+64)%128 partition rotation the rope needs is done
            with a permutation matmul (DVE lanes cannot shift partitions).
            Writes normalized+roped bf16 into out_slice ([128, 512])."""
            sq = tmp_pool.tile([128, 512], BF16, tag="sq")
            nc.vector.tensor_mul(sq, xb, xb)
            scr = psumB.tile([128, 2, 512], F32, tag="scr", name="normscr")
            nc.tensor.matmul(scr[:, 0, :], lhsT=ones_b, rhs=sq, start=True, stop=True)
            nc.tensor.matmul(scr[:, 1, :], lhsT=rot_sb, rhs=xb, start=True, stop=True)
            sqr = tmp_pool.tile([128, 512], F32, tag="sqr", bufs=1)
            nc.scalar.activation(out=sqr, in_=scr[:, 0, :],
                                 func=mybir.ActivationFunctionType.Sqrt,
                                 bias=epst, scale=1.0 / D)
            rs = tmp_pool.tile([128, 512], F32, tag="rs", bufs=1)
            nc.vector.reciprocal_approx_fast(out=rs, in_=sqr)
            t1 = tmp_pool.tile([128, 512], BF16, tag="t1")
            nc.vector.tensor_mul(t1, xb, cst[:, 0, :])
            t2 = tmp_pool.tile([128, 512], BF16, tag="t2")
            nc.vector.tensor_mul(t2, scr[:, 1, :], cst[:, 1, :])
            rsum = tmp_pool.tile([128, 512], BF16, tag="rsum")
            nc.vector.tensor_add(rsum, t1, t2)
            nc.vector.tensor_mul(out_slice, rsum, rs)

        # ================= Q projection =================
        # W-stationary: psum tile [:, h%2, :] = qT of head h for this
        # 512-column group of q positions.
        def q_stage1(g, pq01):
            qb = evac_pool.tile([128, 4, 512], BF16, tag="qb", bufs=2)
            nc.scalar.copy(out=qb[:, 0:2, :], in_=pq01[0])
            nc.scalar.copy(out=qb[:, 2:4, :], in_=pq01[1])
            cst = cs_pool.tile([128, 2, 512], BF16, tag="cs")
            nc.scalar.dma_start(out=cst, in_=csqT[:, g * 512:(g + 1) * 512].rearrange(
                "(two p) c -> p two c", p=128))
            for h in range(HPC):
                norm_rope_T(qb[:, h, :], cst,
                            qT_sb[:, h, g * 512:(g + 1) * 512])
            return None

        for g in range(4):
            pq01 = [psumA.tile([128, 2, 512], F32, tag="acc", name=f"pq{g}_{i}")
                    for i in range(2)]
            for k2 in range(NHD // 2):
                ckq = ck_pool.tile([128, 2, 512], BF16, tag="ck")
                nc.sync.dma_start(
                    out=ckq,
                    in_=ckT[k2 * 256:(k2 + 1) * 256,
                            CTX + g * 512: CTX + (g + 1) * 512].rearrange(
                        "(two p) c -> p two c", p=128))
                for two in range(2):
                    k = 2 * k2 + two
                    for h in range(HPC):
                        nc.tensor.matmul(pq01[h // 2][:, h % 2, :],
                                         lhsT=wq_sb[:, k, h * 128:(h + 1) * 128],
                                         rhs=ckq[:, two, :], start=(k == 0), stop=(k == NHD - 1))
            step_pipeline(lambda g=g, pq01=pq01: q_stage1(g, pq01))

        # ================= K/V projection =================
        # W-stationary: pk[:, 0, :] = K^T, pk[:, 1, :] = V^T for this
        # 512-column group of kv positions.
        def kv_stage1(rq, pk):
            kb = evac_pool.tile([128, 2, 512], BF16, tag="kb")
            nc.scalar.copy(out=kb, in_=pk)
            cst = cs_pool.tile([128, 2, 512], BF16, tag="cs")
            nc.scalar.dma_start(out=cst, in_=cskT[:, rq * 512:(rq + 1) * 512].rearrange(
                "(two p) c -> p two c", p=128))
            norm_rope_T(kb[:, 0, :], cst, kT_sb[:, rq * 512:(rq + 1) * 512])

            def kv_stage2(rq=rq, kb=kb):
                tpv = psumB.tile([128, 512], BF16, tag="scr", name="tpv")
                for rr in range(4):
                    nc.tensor.transpose(tpv[:, rr * 128:(rr + 1) * 128],
                                        kb[:, 1, rr * 128:(rr + 1) * 128], identb)
                nc.scalar.copy(out=v_sb[:, rq * 4:(rq + 1) * 4, :],
                               in_=tpv.rearrange("p (r d) -> p r d", r=4))
            return kv_stage2

        for rq in range(NKV // 4):
            pk = psumA.tile([128, 2, 512], F32, tag="acc", name=f"pk{rq}")
            for k2 in range(NHD // 2):
                ckt = ck_pool.tile([128, 2, 512], BF16, tag="ck")
                nc.sync.dma_start(
                    out=ckt,
                    in_=ckT[k2 * 256:(k2 + 1) * 256,
                            rq * 512:(rq + 1) * 512].rearrange("(two p) c -> p two c", p=128))
                for two in range(2):
                    k = 2 * k2 + two
                    for half in range(2):
                        nc.tensor.matmul(pk[:, half, :],
                                         lhsT=wkv_sb[:, k, half * 128:(half + 1) * 128],
                                         rhs=ckt[:, two, :], start=(k == 0), stop=(k == NHD - 1))
            step_pipeline(lambda rq=rq, pk=pk: kv_stage1(rq, pk))

        step_pipeline()
        step_pipeline()

        # ================= attention =================
        ones_rg = [list(range(NCORES))]

        def emit_oproj(jq):
            for qc in range(4 * jq, 4 * jq + 4):
                qo = (qc % 4) * 128
                po = psumB.tile([128, 512], F32, tag="scr", name="po")
                first = True
                for hp in range(2):
                    for hl in range(2):
                        at = at_pool.tile([128, NCORES, 128], BF16, tag="at")
                        nc.sync.dma_start(
                            out=at,
                            in_=ag_outs[hp][jq][:, hl * 512 + qo: hl * 512 + qo + 128]
                            .rearrange("(c p) q -> p c q", p=128))
                        for ci in range(NCORES):
                            nc.tensor.matmul(po, lhsT=at[:, ci, :],
                                             rhs=wo_sb[:, HPC * ci + 2 * hp + hl, :],
                                             start=first,
                                             stop=(hp == 1 and hl == 1 and ci == NCORES - 1))
                            first = False
                ot = stg_pool.tile([128, 512], F32, tag="ot")
                nc.vector.tensor_copy(out=ot, in_=po)
                nc.scalar.dma_start(out=out_ext[qc * 128:(qc + 1) * 128, :], in_=ot)

        for j in range(NQJ):
            rmax = 35 + 4 * j if j < NQJ - 1 else NKV - 1
            for hp in range(2):
                sacc = sacc_pool.tile([128, 2, 512], F32, tag="sacc", bufs=2)
                nc.vector.memset(sacc, 0.0)
                o_acc = psumB.tile([128, 2, 512], F32, tag="oacc")
                pt_pair = []
                for r in range(rmax + 1):
                    st = psumA.tile([128, 2, 512], F32, tag="acc")
                    for hl in range(2):
                        nc.tensor.matmul(st[:, hl, :], lhsT=kT_sb[:, r * 128:(r + 1) * 128],
                                         rhs=qT_sb[:, 2 * hp + hl, j * 512:(j + 1) * 512],
                                         start=True, stop=True)
                    i = r - 32 - 4 * j
                    if i >= 0:
                        w = 128 * (i + 1)
                        for hl in range(2):
                            nc.vector.tensor_add(st[:, hl, 0:w], st[:, hl, 0:w],
                                                 msk_sb[:, i, 0:w])
                    pt = pt_pool.tile([128, 2, 512], BF16, tag="pt")
                    nc.scalar.activation(out=pt, in_=st,
                                         func=mybir.ActivationFunctionType.Exp,
                                         bias=zbias, scale=SCALE)
                    for hl in range(2):
                        nc.tensor.matmul(o_acc[:, hl, :], lhsT=v_sb[:, r, :],
                                         rhs=pt[:, hl, :],
                                         start=(r == 0), stop=(r == rmax))
                    pt_pair.append(pt)
                    if len(pt_pair) == 2:
                        u = tmp_pool.tile([128, 2, 512], BF16, tag="u")
                        nc.vector.tensor_add(u, pt_pair[0], pt_pair[1])
                        nc.vector.tensor_add(sacc, sacc, u)
                        pt_pair = []
                if pt_pair:
                    nc.vector.tensor_add(sacc, sacc, pt_pair[0])
                    pt_pair = []
                # sweep end: partition-reduce via f32 ones-matmul (tensor),
                # recip, normalize straight out of PSUM, stage + AllGather.
                dps = psumB.tile([128, 2, 512], F32, tag="scr", name="dps")
                for hl in range(2):
                    nc.tensor.matmul(dps[:, hl, :], lhsT=ones_f, rhs=sacc[:, hl, :],
                                     start=True, stop=True)
                pri = sacc_pool.tile([128, 2, 512], F32, tag="pri")
                nc.vector.reciprocal_approx_fast(out=pri, in_=dps)
                stg = stg_pool.tile([128, 2, 512], BF16, tag="stg")
                nc.vector.tensor_mul(stg, o_acc, pri)
                nc.scalar.dma_start(out=ag_ins[hp][j][:], in_=stg.rearrange("p a b -> p (a b)"))
                nc.gpsimd.collective_compute(
                    "AllGather",
                    mybir.AluOpType.bypass,
                    ins=[ag_ins[hp][j][:]],
                    outs=[ag_outs[hp][j][:]],
                    replica_groups=ones_rg,
                )
            if j >= 1:
                emit_oproj(j - 1)
        emit_oproj(NQJ - 1)

    nc.compile()
    return nc


def _host_prep(context, query, w_qkv, w_o, q_norm_w, k_norm_w):
    context = np.asarray(context, dtype=np.float32)
    query = np.asarray(query, dtype=np.float32)
    w_qkv = np.asarray(w_qkv, dtype=np.float32)
    w_o = np.asarray(w_o, dtype=np.float32)
    q_norm_w = np.asarray(q_norm_w, dtype=np.float32)
    k_norm_w = np.asarray(k_norm_w, dtype=np.float32)

    ck = np.concatenate([context, query], axis=0)  # [KV, HID]
    ckT = np.ascontiguousarray(ck.T).astype(bfloat16)  # [HID, KV]

    wq = w_qkv[:, :H * D]
    wk = w_qkv[:, H * D:H * D + KVH * D]
    wv = w_qkv[:, H * D + KVH * D:]

    half = D // 2
    inv_freq = (1.0 / (THETA ** (np.arange(0, half, dtype=np.float32) / half))).astype(np.float32)
    pos = np.arange(KV, dtype=np.float32)
    freqs = pos[:, None] * inv_freq[None, :]   # [KV, 64]
    c = np.cos(freqs).T                        # [64, KV]
    s = np.sin(freqs).T

    def make_csT(nw):
        nw1 = nw[:half, None]
        nw2 = nw[half:, None]
        A = np.concatenate([c * nw1, c * nw2], axis=0)       # [128, KV]
        B = np.concatenate([-s * nw2, s * nw1], axis=0)      # [128, KV]
        return np.concatenate([A, B], axis=0).astype(bfloat16)  # [256, KV]

    cskT_full = make_csT(k_norm_w)
    csqT_full = make_csT(q_norm_w)[:, CTX:]

    p = np.arange(128)[:, None]
    q = np.arange(512)[None, :]
    msk = np.concatenate(
        [np.where(128 * i + p <= q, 0.0, MASKVAL) for i in range(4)],
        axis=1).astype(bfloat16)  # [128, 2048]

    rot = np.zeros((128, 128), dtype=np.float32)
    rot[(np.arange(128) + 64) % 128, np.arange(128)] = 1.0
    rot = rot.astype(bfloat16)

    in_maps = []
    for cidx in range(NCORES):
        in_maps.append({
            "ckT": ckT,
            "wq": np.ascontiguousarray(wq[:, cidx * HPC * D:(cidx + 1) * HPC * D]).astype(bfloat16),
            "wkv": np.ascontiguousarray(
                np.concatenate([wk[:, cidx * D:(cidx + 1) * D], wv[:, cidx * D:(cidx + 1) * D]], axis=1)
            ).astype(bfloat16),
            "wo": np.ascontiguousarray(w_o[:, cidx * HPC * D:(cidx + 1) * HPC * D]).astype(bfloat16),
            "csqT": csqT_full,
            "cskT": cskT_full,
            "msk": msk,
            "rot": rot,
        })
    return in_maps


def kernel(context, query, w_qkv, w_o, q_norm_w, k_norm_w, **kw):
    if "nc" not in _STATE:
        _STATE["nc"] = _build()
    nc = _STATE["nc"]
    in_maps = _host_prep(context, query, w_qkv, w_o, q_norm_w, k_norm_w)
    res = run_bass_kernel_spmd(nc, in_maps, list(range(NCORES)), **kw)
    out = np.concatenate([np.asarray(res.results[c]["out"]) for c in range(NCORES)], axis=1)
    if kw:
        return out.astype(np.float32), res
    return out.astype(np.float32)


# revision 22
# speedup vs baseline: 1.2724x; 1.0119x over previous
"""DFlash Qwen3 cross-attention on 8 TRN2 NeuronCores.

Sharding: tensor-parallel over heads. Core c owns KV head c (KVH=8) and the
4 query heads 4c..4c+3 of its GQA group.

v4 structure (evolved from baseline/v2/v3 trace analysis):
- All DMAs on HWDGE rings (sync = big streams + o_proj loads, scalar =
  cos/sin loads, AG staging, output stores).  No SWDGE descriptor-gen.
- Both projections are W-stationary: lhsT = weight chunk, moving = ckT
  columns at N=512, so Q^T and K^T come out of PSUM directly in the
  [d, pos] layout attention wants (no per-chunk PE transposes) and the
  KV matmul count halves vs the ck-stationary form.  V^T is transposed
  back to natural via 48 PE transposes.
- RMSNorm in transposed space: evac PSUM->bf16 SBUF, square (DVE 2x),
  partition-sum via a bf16 ones-matmul on the tensor engine (output is
  broadcast over partitions), ACT sqrt + reciprocal_approx_fast, rope
  via two host-precomputed transposed cos/sin tiles (norm weight folded
  in), final per-column 1/rms multiply writes qT/kT directly.
- Attention: j outermost (AG -> o_proj one j-tile behind), head pairs
  share kT/v stationaries, ONE [128,1024] exp per r-step, softmax
  denominator on vector only via bf16 pair-sums + f32 accumulate.
- Sweep end: partition-reduce of the denominator via an f32 ones-matmul
  (tensor), reciprocal_approx_fast, normalize straight out of PSUM.
  gpsimd runs ONLY the collective triggers: the AllGather trigger blocks
  its queue until the collective completes (~20us), which in v3 starved
  partition_all_reduce and stalled the whole pipeline at j boundaries.
"""

from contextlib import ExitStack

import numpy as np
from ml_dtypes import bfloat16

import concourse.bass as bass
import concourse.bass_isa as bass_isa
import concourse.mybir as mybir
import concourse.tile as tile
from concourse import bacc
from concourse.bass_utils import run_bass_kernel_spmd
from concourse.masks import make_identity

H = 32
KVH = 8
D = 128
HID = 4096
CTX = 4096
QL = 2048
KV = CTX + QL  # 6144
NCORES = 8
HPC = H // NCORES  # 4 q heads per core
THETA = 1000000.0
EPS = 1e-6
SCALE = float(D) ** -0.5

NHD = HID // 128  # 32 contraction chunks
NKV = KV // 128  # 48 kv chunks
NQC = QL // 128  # 16 q row chunks
NQJ = QL // 512  # 4 q column tiles for attention
MASKVAL = -1e6

F32 = mybir.dt.float32
BF16 = mybir.dt.bfloat16
MULT = mybir.AluOpType.mult

_STATE = {}


def _build():
    nc = bacc.Bacc()

    ckT = nc.declare_dram_parameter("ckT", [HID, KV], BF16, isOutput=False)
    wq = nc.declare_dram_parameter("wq", [HID, HPC * D], BF16, isOutput=False)
    wkv = nc.declare_dram_parameter("wkv", [HID, 2 * D], BF16, isOutput=False)
    wo = nc.declare_dram_parameter("wo", [HID, HPC * D], BF16, isOutput=False)
    csqT = nc.declare_dram_parameter("csqT", [2 * D, QL], BF16, isOutput=False)
    cskT = nc.declare_dram_parameter("cskT", [2 * D, KV], BF16, isOutput=False)
    msk = nc.declare_dram_parameter("msk", [128, 4 * 512], BF16, isOutput=False)
    rot = nc.declare_dram_parameter("rot", [128, 128], BF16, isOutput=False)
    out_ext = nc.declare_dram_parameter("out", [QL, HPC * D], F32, isOutput=True)

    ag_ins = [[nc.dram_tensor(f"ag_in{hp}_{j}", [128, 1024], BF16) for j in range(NQJ)]
              for hp in range(2)]
    ag_outs = [[nc.dram_tensor(f"ag_out{hp}_{j}", [NCORES * 128, 1024], BF16,
                               addr_space="Shared") for j in range(NQJ)]
               for hp in range(2)]

    with tile.TileContext(nc) as tc, ExitStack() as ctx:
        singles = ctx.enter_context(tc.tile_pool(name="singles", bufs=1))
        ck_pool = ctx.enter_context(tc.tile_pool(name="ckp", bufs=4))
        cs_pool = ctx.enter_context(tc.tile_pool(name="csp", bufs=3))
        evac_pool = ctx.enter_context(tc.tile_pool(name="evac", bufs=3))
        tmp_pool = ctx.enter_context(tc.tile_pool(name="tmp", bufs=2))
        pt_pool = ctx.enter_context(tc.tile_pool(name="ptp", bufs=4))
        sacc_pool = ctx.enter_context(tc.tile_pool(name="sacc", bufs=1))
        stg_pool = ctx.enter_context(tc.tile_pool(name="stg", bufs=2))
        at_pool = ctx.enter_context(tc.tile_pool(name="atp", bufs=2))
        # PSUM: A = proj accumulators / attention S^T (2 x 2 banks = 4);
        # B: "oacc" (2 banks) + "scr" (2 banks: proj norm-sums + V
        # transposes, attention denominator, o_proj accumulator)
        psumA = ctx.enter_context(tc.tile_pool(name="psA", bufs=2, space="PSUM"))
        psumB = ctx.enter_context(tc.tile_pool(name="psB", bufs=1, space="PSUM"))

        # ---- resident tensors ----
        wq_sb = singles.tile([128, NHD, HPC * D], BF16)
        for wpc in range(8):
            kk = NHD // 8
            nc.sync.dma_start(
                out=wq_sb[:, wpc * kk:(wpc + 1) * kk, :],
                in_=wq[wpc * kk * 128:(wpc + 1) * kk * 128, :].rearrange(
                    "(k p) n -> p k n", p=128))
        wkv_sb = singles.tile([128, NHD, 2 * D], BF16)
        for wpc in range(2):
            kk = NHD // 2
            nc.sync.dma_start(
                out=wkv_sb[:, wpc * kk:(wpc + 1) * kk, :],
                in_=wkv[wpc * kk * 128:(wpc + 1) * kk * 128, :].rearrange(
                    "(k p) n -> p k n", p=128))
        wo_sb = singles.tile([128, NHD, HPC * D], BF16)
        nc.scalar.dma_start(out=wo_sb[:], in_=wo[:, :].rearrange("(k p) n -> p k n", p=128))
        msk_sb = singles.tile([128, 4, 512], BF16)
        nc.scalar.dma_start(out=msk_sb[:], in_=msk[:, :].rearrange("p (i c) -> p i c", i=4))

        identb = singles.tile([128, 128], BF16)
        make_identity(nc, identb)
        rot_sb = singles.tile([128, 128], BF16)
        nc.scalar.dma_start(out=rot_sb, in_=rot[:, :])
        ones_b = singles.tile([128, 128], BF16)
        nc.vector.memset(ones_b, 1.0)
        ones_f = singles.tile([128, 128], F32)
        nc.vector.memset(ones_f, 1.0)
        epst = singles.tile([128, 1], F32)
        nc.vector.memset(epst, EPS)
        zbias = singles.tile([128, 1], F32)
        nc.vector.memset(zbias, 0.0)

        qT_sb = singles.tile([128, HPC, QL], BF16)  # Q^T per head: [d, h, q]
        kT_sb = singles.tile([128, KV], BF16)  # K^T: [d, kv]
        v_sb = singles.tile([128, NKV, D], BF16)  # V: [kv%128, r, d]

        pend1 = [None]
        pend2 = [None]

        def step_pipeline(new_stage1=None):
            if pend2[0] is not None:
                pend2[0]()
            pend2[0] = None
            if pend1[0] is not None:
                pend2[0] = pend1[0]()
            pend1[0] = new_stage1

        def norm_rope_T(xb, cst, out_slice):
            """Transposed-space rmsnorm+rope for one [128, 512] tile.

            xb: [128, 512] bf16 (rows = d, cols = positions).
            cst: [128, 2, 512] bf16 (A_T, B_T with norm weight folded in).
            The d -> (d+64)%128 partition rotation the rope needs is done
            with a permutation matmul (DVE lanes cannot shift partitions).
            Writes normalized+roped bf16 into out_slice ([128, 512])."""
            sq = tmp_pool.tile([128, 512], BF16, tag="sq")
            nc.vector.tensor_mul(sq, xb, xb)
            scr = psumB.tile([128, 2, 512], F32, tag="scr", name="normscr")
            nc.tensor.matmul(scr[:, 0, :], lhsT=ones_b, rhs=sq, start=True, stop=True)
            nc.tensor.matmul(scr[:, 1, :], lhsT=rot_sb, rhs=xb, start=True, stop=True)
            sqr = tmp_pool.tile([128, 512], F32, tag="sqr", bufs=1)
            nc.scalar.activation(out=sqr, in_=scr[:, 0, :],
                                 func=mybir.ActivationFunctionType.Sqrt,
                                 bias=epst, scale=1.0 / D)
            rs = tmp_pool.tile([128, 512], F32, tag="rs", bufs=1)
            nc.vector.reciprocal_approx_fast(out=rs, in_=sqr)
            t1 = tmp_pool.tile([128, 512], BF16, tag="t1")
            nc.vector.tensor_mul(t1, xb, cst[:, 0, :])
            t2 = tmp_pool.tile([128, 512], BF16, tag="t2")
            nc.vector.tensor_mul(t2, scr[:, 1, :], cst[:, 1, :])
            rsum = tmp_pool.tile([128, 512], BF16, tag="rsum")
            nc.vector.tensor_add(rsum, t1, t2)
            nc.vector.tensor_mul(out_slice, rsum, rs)

        # ================= Q projection =================
        # W-stationary: psum tile [:, h%2, :] = qT of head h for this
        # 512-column group of q positions.
        def q_stage1(g, pq01):
            qb = evac_pool.tile([128, 4, 512], BF16, tag="qb", bufs=2)
            nc.scalar.copy(out=qb[:, 0:2, :], in_=pq01[0])
            nc.scalar.copy(out=qb[:, 2:4, :], in_=pq01[1])
            cst = cs_pool.tile([128, 2, 512], BF16, tag="cs")
            nc.scalar.dma_start(out=cst, in_=csqT[:, g * 512:(g + 1) * 512].rearrange(
                "(two p) c -> p two c", p=128))
            for h in range(HPC):
                norm_rope_T(qb[:, h, :], cst,
                            qT_sb[:, h, g * 512:(g + 1) * 512])
            return None

        for g in range(4):
            pq01 = [psumA.tile([128, 2, 512], F32, tag="acc", name=f"pq{g}_{i}")
                    for i in range(2)]
            for k2 in range(NHD // 2):
                ckq = ck_pool.tile([128, 2, 512], BF16, tag="ck")
                nc.sync.dma_start(
                    out=ckq,
                    in_=ckT[k2 * 256:(k2 + 1) * 256,
                            CTX + g * 512: CTX + (g + 1) * 512].rearrange(
                        "(two p) c -> p two c", p=128))
                for two in range(2):
                    k = 2 * k2 + two
                    for h in range(HPC):
                        nc.tensor.matmul(pq01[h // 2][:, h % 2, :],
                                         lhsT=wq_sb[:, k, h * 128:(h + 1) * 128],
                                         rhs=ckq[:, two, :], start=(k == 0), stop=(k == NHD - 1))
            step_pipeline(lambda g=g, pq01=pq01: q_stage1(g, pq01))

        # ================= K/V projection =================
        # W-stationary: pk[:, 0, :] = K^T, pk[:, 1, :] = V^T for this
        # 512-column group of kv positions.
        def kv_stage1(rq, pk):
            kb = evac_pool.tile([128, 2, 512], BF16, tag="kb")
            nc.scalar.copy(out=kb, in_=pk)
            cst = cs_pool.tile([128, 2, 512], BF16, tag="cs")
            nc.scalar.dma_start(out=cst, in_=cskT[:, rq * 512:(rq + 1) * 512].rearrange(
                "(two p) c -> p two c", p=128))
            norm_rope_T(kb[:, 0, :], cst, kT_sb[:, rq * 512:(rq + 1) * 512])

            def kv_stage2(rq=rq, kb=kb):
                tpv = psumB.tile([128, 512], BF16, tag="scr", name="tpv")
                for rr in range(4):
                    nc.tensor.transpose(tpv[:, rr * 128:(rr + 1) * 128],
                                        kb[:, 1, rr * 128:(rr + 1) * 128], identb)
                nc.scalar.copy(out=v_sb[:, rq * 4:(rq + 1) * 4, :],
                               in_=tpv.rearrange("p (r d) -> p r d", r=4))
            return kv_stage2

        for rq in range(NKV // 4):
            pk = psumA.tile([128, 2, 512], F32, tag="acc", name=f"pk{rq}")
            for k2 in range(NHD // 2):
                ckt = ck_pool.tile([128, 2, 512], BF16, tag="ck")
                nc.sync.dma_start(
                    out=ckt,
                    in_=ckT[k2 * 256:(k2 + 1) * 256,
                            rq * 512:(rq + 1) * 512].rearrange("(two p) c -> p two c", p=128))
                for two in range(2):
                    k = 2 * k2 + two
                    for half in range(2):
                        nc.tensor.matmul(pk[:, half, :],
                                         lhsT=wkv_sb[:, k, half * 128:(half + 1) * 128],
                                         rhs=ckt[:, two, :], start=(k == 0), stop=(k == NHD - 1))
            step_pipeline(lambda rq=rq, pk=pk: kv_stage1(rq, pk))

        step_pipeline()
        step_pipeline()

        # ================= attention =================
        ones_rg = [list(range(NCORES))]

        def emit_oproj(jq):
            for qc in range(4 * jq, 4 * jq + 4):
                qo = (qc % 4) * 128
                po = psumB.tile([128, 512], F32, tag="scr", name="po")
                first = True
                for hp in range(2):
                    for hl in range(2):
                        at = at_pool.tile([128, NCORES, 128], BF16, tag="at")
                        nc.sync.dma_start(
                            out=at,
                            in_=ag_outs[hp][jq][:, hl * 512 + qo: hl * 512 + qo + 128]
                            .rearrange("(c p) q -> p c q", p=128))
                        for ci in range(NCORES):
                            nc.tensor.matmul(po, lhsT=at[:, ci, :],
                                             rhs=wo_sb[:, HPC * ci + 2 * hp + hl, :],
                                             start=first,
                                             stop=(hp == 1 and hl == 1 and ci == NCORES - 1))
                            first = False
                ot = stg_pool.tile([128, 512], F32, tag="ot")
                nc.vector.tensor_copy(out=ot, in_=po)
                nc.scalar.dma_start(out=out_ext[qc * 128:(qc + 1) * 128, :], in_=ot)

        for j in range(NQJ):
            rmax = 35 + 4 * j if j < NQJ - 1 else NKV - 1
            for hp in range(2):
                sacc = sacc_pool.tile([128, 2, 512], F32, tag="sacc", bufs=2)
                nc.vector.memset(sacc, 0.0)
                o_acc = psumB.tile([128, 2, 512], F32, tag="oacc")
                pt_pair = []
                pv_q = []
                for r in range(rmax + 1):
                    st = psumA.tile([128, 2, 512], F32, tag="acc")
                    for hl in range(2):
                        nc.tensor.matmul(st[:, hl, :], lhsT=kT_sb[:, r * 128:(r + 1) * 128],
                                         rhs=qT_sb[:, 2 * hp + hl, j * 512:(j + 1) * 512],
                                         start=True, stop=True)
                    i = r - 32 - 4 * j
                    if i >= 0:
                        w = 128 * (i + 1)
                        for hl in range(2):
                            nc.vector.tensor_add(st[:, hl, 0:w], st[:, hl, 0:w],
                                                 msk_sb[:, i, 0:w])
                    pt = pt_pool.tile([128, 2, 512], BF16, tag="pt")
                    nc.scalar.activation(out=pt, in_=st,
                                         func=mybir.ActivationFunctionType.Exp,
                                         bias=zbias, scale=SCALE)
                    # PV emitted one r behind so it never waits on the exp
                    pv_q.append((r, pt))
                    if len(pv_q) == 2:
                        rr, ptp = pv_q.pop(0)
                        for hl in range(2):
                            nc.tensor.matmul(o_acc[:, hl, :], lhsT=v_sb[:, rr, :],
                                             rhs=ptp[:, hl, :],
                                             start=(rr == 0), stop=False)
                    pt_pair.append(pt)
                    if len(pt_pair) == 2:
                        u = tmp_pool.tile([128, 2, 512], BF16, tag="u", bufs=1)
                        nc.vector.tensor_add(u, pt_pair[0], pt_pair[1])
                        nc.vector.tensor_add(sacc, sacc, u)
                        pt_pair = []
                if pt_pair:
                    nc.vector.tensor_add(sacc, sacc, pt_pair[0])
                    pt_pair = []
                rr, ptp = pv_q.pop(0)
                for hl in range(2):
                    nc.tensor.matmul(o_acc[:, hl, :], lhsT=v_sb[:, rr, :],
                                     rhs=ptp[:, hl, :],
                                     start=(rr == 0), stop=True)
                # sweep end: free o_acc promptly with a scalar evac, then
                # partition-reduce via f32 ones-matmul, recip, normalize.
                oraw = sacc_pool.tile([128, 2, 512], F32, tag="oraw")
                nc.scalar.copy(out=oraw, in_=o_acc)
                dps = psumB.tile([128, 2, 512], F32, tag="scr", name="dps")
                for hl in range(2):
                    nc.tensor.matmul(dps[:, hl, :], lhsT=ones_f, rhs=sacc[:, hl, :],
                                     start=True, stop=True)
                pri = sacc_pool.tile([128, 2, 512], F32, tag="pri")
                nc.vector.reciprocal_approx_fast(out=pri, in_=dps)
                stg = stg_pool.tile([128, 2, 512], BF16, tag="stg")
                nc.vector.tensor_mul(stg, oraw, pri)
                nc.scalar.dma_start(out=ag_ins[hp][j][:], in_=stg.rearrange("p a b -> p (a b)"))
                nc.gpsimd.collective_compute(
                    "AllGather",
                    mybir.AluOpType.bypass,
                    ins=[ag_ins[hp][j][:]],
                    outs=[ag_outs[hp][j][:]],
                    replica_groups=ones_rg,
                )
            if j >= 1:
                emit_oproj(j - 1)
        emit_oproj(NQJ - 1)

    nc.compile()
    return nc


def _host_prep(context, query, w_qkv, w_o, q_norm_w, k_norm_w):
    context = np.asarray(context, dtype=np.float32)
    query = np.asarray(query, dtype=np.float32)
    w_qkv = np.asarray(w_qkv, dtype=np.float32)
    w_o = np.asarray(w_o, dtype=np.float32)
    q_norm_w = np.asarray(q_norm_w, dtype=np.float32)
    k_norm_w = np.asarray(k_norm_w, dtype=np.float32)

    ck = np.concatenate([context, query], axis=0)  # [KV, HID]
    ckT = np.ascontiguousarray(ck.T).astype(bfloat16)  # [HID, KV]

    wq = w_qkv[:, :H * D]
    wk = w_qkv[:, H * D:H * D + KVH * D]
    wv = w_qkv[:, H * D + KVH * D:]

    half = D // 2
    inv_freq = (1.0 / (THETA ** (np.arange(0, half, dtype=np.float32) / half))).astype(np.float32)
    pos = np.arange(KV, dtype=np.float32)
    freqs = pos[:, None] * inv_freq[None, :]   # [KV, 64]
    c = np.cos(freqs).T                        # [64, KV]
    s = np.sin(freqs).T

    def make_csT(nw):
        nw1 = nw[:half, None]
        nw2 = nw[half:, None]
        A = np.concatenate([c * nw1, c * nw2], axis=0)       # [128, KV]
        B = np.concatenate([-s * nw2, s * nw1], axis=0)      # [128, KV]
        return np.concatenate([A, B], axis=0).astype(bfloat16)  # [256, KV]

    cskT_full = make_csT(k_norm_w)
    csqT_full = make_csT(q_norm_w)[:, CTX:]

    p = np.arange(128)[:, None]
    q = np.arange(512)[None, :]
    msk = np.concatenate(
        [np.where(128 * i + p <= q, 0.0, MASKVAL) for i in range(4)],
        axis=1).astype(bfloat16)  # [128, 2048]

    rot = np.zeros((128, 128), dtype=np.float32)
    rot[(np.arange(128) + 64) % 128, np.arange(128)] = 1.0
    rot = rot.astype(bfloat16)

    in_maps = []
    for cidx in range(NCORES):
        in_maps.append({
            "ckT": ckT,
            "wq": np.ascontiguousarray(wq[:, cidx * HPC * D:(cidx + 1) * HPC * D]).astype(bfloat16),
            "wkv": np.ascontiguousarray(
                np.concatenate([wk[:, cidx * D:(cidx + 1) * D], wv[:, cidx * D:(cidx + 1) * D]], axis=1)
            ).astype(bfloat16),
            "wo": np.ascontiguousarray(w_o[:, cidx * HPC * D:(cidx + 1) * HPC * D]).astype(bfloat16),
            "csqT": csqT_full,
            "cskT": cskT_full,
            "msk": msk,
            "rot": rot,
        })
    return in_maps


def kernel(context, query, w_qkv, w_o, q_norm_w, k_norm_w, **kw):
    if "nc" not in _STATE:
        _STATE["nc"] = _build()
    nc = _STATE["nc"]
    in_maps = _host_prep(context, query, w_qkv, w_o, q_norm_w, k_norm_w)
    res = run_bass_kernel_spmd(nc, in_maps, list(range(NCORES)), **kw)
    out = np.concatenate([np.asarray(res.results[c]["out"]) for c in range(NCORES)], axis=1)
    if kw:
        return out.astype(np.float32), res
    return out.astype(np.float32)


# revision 23
# speedup vs baseline: 1.2794x; 1.0055x over previous
"""DFlash Qwen3 cross-attention on 8 TRN2 NeuronCores.

Sharding: tensor-parallel over heads. Core c owns KV head c (KVH=8) and the
4 query heads 4c..4c+3 of its GQA group.

v4 structure (evolved from baseline/v2/v3 trace analysis):
- All DMAs on HWDGE rings (sync = big streams + o_proj loads, scalar =
  cos/sin loads, AG staging, output stores).  No SWDGE descriptor-gen.
- Both projections are W-stationary: lhsT = weight chunk, moving = ckT
  columns at N=512, so Q^T and K^T come out of PSUM directly in the
  [d, pos] layout attention wants (no per-chunk PE transposes) and the
  KV matmul count halves vs the ck-stationary form.  V^T is transposed
  back to natural via 48 PE transposes.
- RMSNorm in transposed space: evac PSUM->bf16 SBUF, square (DVE 2x),
  partition-sum via a bf16 ones-matmul on the tensor engine (output is
  broadcast over partitions), ACT sqrt + reciprocal_approx_fast, rope
  via two host-precomputed transposed cos/sin tiles (norm weight folded
  in), final per-column 1/rms multiply writes qT/kT directly.
- Attention: j outermost (AG -> o_proj one j-tile behind), head pairs
  share kT/v stationaries, ONE [128,1024] exp per r-step, softmax
  denominator on vector only via bf16 pair-sums + f32 accumulate.
- Sweep end: partition-reduce of the denominator via an f32 ones-matmul
  (tensor), reciprocal_approx_fast, normalize straight out of PSUM.
  gpsimd runs ONLY the collective triggers: the AllGather trigger blocks
  its queue until the collective completes (~20us), which in v3 starved
  partition_all_reduce and stalled the whole pipeline at j boundaries.
"""

from contextlib import ExitStack

import numpy as np
from ml_dtypes import bfloat16

import concourse.bass as bass
import concourse.bass_isa as bass_isa
import concourse.mybir as mybir
import concourse.tile as tile
from concourse import bacc
from concourse.bass_utils import run_bass_kernel_spmd
from concourse.masks import make_identity

H = 32
KVH = 8
D = 128
HID = 4096
CTX = 4096
QL = 2048
KV = CTX + QL  # 6144
NCORES = 8
HPC = H // NCORES  # 4 q heads per core
THETA = 1000000.0
EPS = 1e-6
SCALE = float(D) ** -0.5

NHD = HID // 128  # 32 contraction chunks
NKV = KV // 128  # 48 kv chunks
NQC = QL // 128  # 16 q row chunks
NQJ = QL // 512  # 4 q column tiles for attention
MASKVAL = -1e6

F32 = mybir.dt.float32
BF16 = mybir.dt.bfloat16
MULT = mybir.AluOpType.mult

_STATE = {}


def _build():
    nc = bacc.Bacc()

    ckT = nc.declare_dram_parameter("ckT", [HID, KV], BF16, isOutput=False)
    wq = nc.declare_dram_parameter("wq", [HID, HPC * D], BF16, isOutput=False)
    wkv = nc.declare_dram_parameter("wkv", [HID, 2 * D], BF16, isOutput=False)
    wo = nc.declare_dram_parameter("wo", [HID, HPC * D], BF16, isOutput=False)
    csqT = nc.declare_dram_parameter("csqT", [2 * D, QL], BF16, isOutput=False)
    cskT = nc.declare_dram_parameter("cskT", [2 * D, KV], BF16, isOutput=False)
    msk = nc.declare_dram_parameter("msk", [128, 4 * 512], BF16, isOutput=False)
    rot = nc.declare_dram_parameter("rot", [128, 128], BF16, isOutput=False)
    out_ext = nc.declare_dram_parameter("out", [QL, HPC * D], F32, isOutput=True)

    ag_ins = [[nc.dram_tensor(f"ag_in{hp}_{j}", [128, 1024], BF16) for j in range(NQJ)]
              for hp in range(2)]
    ag_outs = [[nc.dram_tensor(f"ag_out{hp}_{j}", [NCORES * 128, 1024], BF16,
                               addr_space="Shared") for j in range(NQJ)]
               for hp in range(2)]

    with tile.TileContext(nc) as tc, ExitStack() as ctx:
        singles = ctx.enter_context(tc.tile_pool(name="singles", bufs=1))
        ck_pool = ctx.enter_context(tc.tile_pool(name="ckp", bufs=4))
        cs_pool = ctx.enter_context(tc.tile_pool(name="csp", bufs=3))
        evac_pool = ctx.enter_context(tc.tile_pool(name="evac", bufs=3))
        tmp_pool = ctx.enter_context(tc.tile_pool(name="tmp", bufs=2))
        pt_pool = ctx.enter_context(tc.tile_pool(name="ptp", bufs=4))
        sacc_pool = ctx.enter_context(tc.tile_pool(name="sacc", bufs=1))
        stg_pool = ctx.enter_context(tc.tile_pool(name="stg", bufs=2))
        at_pool = ctx.enter_context(tc.tile_pool(name="atp", bufs=2))
        # PSUM: A = proj accumulators / attention S^T (2 x 2 banks = 4);
        # B: "oacc" (2 banks) + "scr" (2 banks: proj norm-sums + V
        # transposes, attention denominator, o_proj accumulator)
        psumA = ctx.enter_context(tc.tile_pool(name="psA", bufs=2, space="PSUM"))
        psumB = ctx.enter_context(tc.tile_pool(name="psB", bufs=1, space="PSUM"))

        # ---- resident tensors ----
        wq_sb = singles.tile([128, NHD, HPC * D], BF16)
        for wpc in range(8):
            kk = NHD // 8
            nc.sync.dma_start(
                out=wq_sb[:, wpc * kk:(wpc + 1) * kk, :],
                in_=wq[wpc * kk * 128:(wpc + 1) * kk * 128, :].rearrange(
                    "(k p) n -> p k n", p=128))
        wkv_sb = singles.tile([128, NHD, 2 * D], BF16)
        for wpc in range(2):
            kk = NHD // 2
            nc.sync.dma_start(
                out=wkv_sb[:, wpc * kk:(wpc + 1) * kk, :],
                in_=wkv[wpc * kk * 128:(wpc + 1) * kk * 128, :].rearrange(
                    "(k p) n -> p k n", p=128))
        wo_sb = singles.tile([128, NHD, HPC * D], BF16)
        nc.scalar.dma_start(out=wo_sb[:], in_=wo[:, :].rearrange("(k p) n -> p k n", p=128))
        msk_sb = singles.tile([128, 4, 512], BF16)
        nc.scalar.dma_start(out=msk_sb[:], in_=msk[:, :].rearrange("p (i c) -> p i c", i=4))

        identb = singles.tile([128, 128], BF16)
        make_identity(nc, identb)
        rot_sb = singles.tile([128, 128], BF16)
        nc.scalar.dma_start(out=rot_sb, in_=rot[:, :])
        ones_b = singles.tile([128, 128], BF16)
        nc.vector.memset(ones_b, 1.0)
        ones_f = singles.tile([128, 128], F32)
        nc.vector.memset(ones_f, 1.0)
        epst = singles.tile([128, 1], F32)
        nc.vector.memset(epst, EPS)
        zbias = singles.tile([128, 1], F32)
        nc.vector.memset(zbias, 0.0)

        qT_sb = singles.tile([128, HPC, QL], BF16)  # Q^T per head: [d, h, q]
        kT_sb = singles.tile([128, KV], BF16)  # K^T: [d, kv]
        v_sb = singles.tile([128, NKV, D], BF16)  # V: [kv%128, r, d]

        pend1 = [None]
        pend2 = [None]

        def step_pipeline(new_stage1=None):
            if pend2[0] is not None:
                pend2[0]()
            pend2[0] = None
            if pend1[0] is not None:
                pend2[0] = pend1[0]()
            pend1[0] = new_stage1

        def norm_rope_T(xb, cst, out_slice):
            """Transposed-space rmsnorm+rope for one [128, 512] tile.

            xb: [128, 512] bf16 (rows = d, cols = positions).
            cst: [128, 2, 512] bf16 (A_T, B_T with norm weight folded in).
            The d -> (d+64)%128 partition rotation the rope needs is done
            with a permutation matmul (DVE lanes cannot shift partitions).
            Writes normalized+roped bf16 into out_slice ([128, 512])."""
            sq = tmp_pool.tile([128, 512], BF16, tag="sq")
            nc.vector.tensor_mul(sq, xb, xb)
            scr = psumB.tile([128, 2, 512], F32, tag="scr", name="normscr")
            nc.tensor.matmul(scr[:, 0, :], lhsT=ones_b, rhs=sq, start=True, stop=True)
            nc.tensor.matmul(scr[:, 1, :], lhsT=rot_sb, rhs=xb, start=True, stop=True)
            sqr = tmp_pool.tile([128, 512], F32, tag="sqr", bufs=1)
            nc.scalar.activation(out=sqr, in_=scr[:, 0, :],
                                 func=mybir.ActivationFunctionType.Sqrt,
                                 bias=epst, scale=1.0 / D)
            rs = tmp_pool.tile([128, 512], F32, tag="rs", bufs=1)
            nc.vector.reciprocal_approx_fast(out=rs, in_=sqr)
            t1 = tmp_pool.tile([128, 512], BF16, tag="t1")
            nc.vector.tensor_mul(t1, xb, cst[:, 0, :])
            t2 = tmp_pool.tile([128, 512], BF16, tag="t2")
            nc.vector.tensor_mul(t2, scr[:, 1, :], cst[:, 1, :])
            rsum = tmp_pool.tile([128, 512], BF16, tag="rsum")
            nc.vector.tensor_add(rsum, t1, t2)
            nc.vector.tensor_mul(out_slice, rsum, rs)

        # ================= Q projection =================
        # W-stationary: psum tile [:, h%2, :] = qT of head h for this
        # 512-column group of q positions.
        def q_stage1(g, pq01):
            qb = evac_pool.tile([128, 4, 512], BF16, tag="qb", bufs=2)
            nc.scalar.copy(out=qb[:, 0:2, :], in_=pq01[0])
            nc.scalar.copy(out=qb[:, 2:4, :], in_=pq01[1])
            cst = cs_pool.tile([128, 2, 512], BF16, tag="cs")
            nc.scalar.dma_start(out=cst, in_=csqT[:, g * 512:(g + 1) * 512].rearrange(
                "(two p) c -> p two c", p=128))
            for h in range(HPC):
                norm_rope_T(qb[:, h, :], cst,
                            qT_sb[:, h, g * 512:(g + 1) * 512])
            return None

        for g in range(4):
            pq01 = [psumA.tile([128, 2, 512], F32, tag="acc", name=f"pq{g}_{i}")
                    for i in range(2)]
            for k2 in range(NHD // 2):
                ckq = ck_pool.tile([128, 2, 512], BF16, tag="ck")
                nc.sync.dma_start(
                    out=ckq,
                    in_=ckT[k2 * 256:(k2 + 1) * 256,
                            CTX + g * 512: CTX + (g + 1) * 512].rearrange(
                        "(two p) c -> p two c", p=128))
                for two in range(2):
                    k = 2 * k2 + two
                    for h in range(HPC):
                        nc.tensor.matmul(pq01[h // 2][:, h % 2, :],
                                         lhsT=wq_sb[:, k, h * 128:(h + 1) * 128],
                                         rhs=ckq[:, two, :], start=(k == 0), stop=(k == NHD - 1))
            step_pipeline(lambda g=g, pq01=pq01: q_stage1(g, pq01))

        # ================= K/V projection =================
        # W-stationary: pk[:, 0, :] = K^T, pk[:, 1, :] = V^T for this
        # 512-column group of kv positions.
        def kv_stage1(rq, pk):
            kb = evac_pool.tile([128, 2, 512], BF16, tag="kb")
            nc.scalar.copy(out=kb, in_=pk)
            cst = cs_pool.tile([128, 2, 512], BF16, tag="cs")
            nc.scalar.dma_start(out=cst, in_=cskT[:, rq * 512:(rq + 1) * 512].rearrange(
                "(two p) c -> p two c", p=128))
            norm_rope_T(kb[:, 0, :], cst, kT_sb[:, rq * 512:(rq + 1) * 512])

            def kv_stage2(rq=rq, kb=kb):
                tpv = psumB.tile([128, 512], BF16, tag="scr", name="tpv")
                for rr in range(4):
                    nc.tensor.transpose(tpv[:, rr * 128:(rr + 1) * 128],
                                        kb[:, 1, rr * 128:(rr + 1) * 128], identb)
                nc.scalar.copy(out=v_sb[:, rq * 4:(rq + 1) * 4, :],
                               in_=tpv.rearrange("p (r d) -> p r d", r=4))
            return kv_stage2

        for rq in range(NKV // 4):
            pk = psumA.tile([128, 2, 512], F32, tag="acc", name=f"pk{rq}")
            for k2 in range(NHD // 2):
                ckt = ck_pool.tile([128, 2, 512], BF16, tag="ck")
                nc.sync.dma_start(
                    out=ckt,
                    in_=ckT[k2 * 256:(k2 + 1) * 256,
                            rq * 512:(rq + 1) * 512].rearrange("(two p) c -> p two c", p=128))
                for two in range(2):
                    k = 2 * k2 + two
                    for half in range(2):
                        nc.tensor.matmul(pk[:, half, :],
                                         lhsT=wkv_sb[:, k, half * 128:(half + 1) * 128],
                                         rhs=ckt[:, two, :], start=(k == 0), stop=(k == NHD - 1))
            step_pipeline(lambda rq=rq, pk=pk: kv_stage1(rq, pk))

        step_pipeline()
        step_pipeline()

        # ================= attention =================
        ones_rg = [list(range(NCORES))]

        def emit_oproj(jq):
            for qc in range(4 * jq, 4 * jq + 4):
                qo = (qc % 4) * 128
                po = psumB.tile([128, 512], F32, tag="scr", name="po")
                first = True
                for hp in range(2):
                    for hl in range(2):
                        at = at_pool.tile([128, NCORES, 128], BF16, tag="at")
                        nc.sync.dma_start(
                            out=at,
                            in_=ag_outs[hp][jq][:, hl * 512 + qo: hl * 512 + qo + 128]
                            .rearrange("(c p) q -> p c q", p=128))
                        for ci in range(NCORES):
                            nc.tensor.matmul(po, lhsT=at[:, ci, :],
                                             rhs=wo_sb[:, HPC * ci + 2 * hp + hl, :],
                                             start=first,
                                             stop=(hp == 1 and hl == 1 and ci == NCORES - 1))
                            first = False
                ot = stg_pool.tile([128, 512], F32, tag="ot")
                nc.vector.tensor_copy(out=ot, in_=po)
                nc.scalar.dma_start(out=out_ext[qc * 128:(qc + 1) * 128, :], in_=ot)

        for j in range(NQJ):
            rmax = 35 + 4 * j if j < NQJ - 1 else NKV - 1
            for hp in range(2):
                sacc = sacc_pool.tile([128, 2, 512], F32, tag="sacc", bufs=2)
                nc.vector.memset(sacc, 0.0)
                o_acc = psumB.tile([128, 2, 512], F32, tag="oacc")
                pt_pair = []
                pv_q = []
                for r in range(rmax + 1):
                    st = psumA.tile([128, 2, 512], F32, tag="acc")
                    for hl in range(2):
                        nc.tensor.matmul(st[:, hl, :], lhsT=kT_sb[:, r * 128:(r + 1) * 128],
                                         rhs=qT_sb[:, 2 * hp + hl, j * 512:(j + 1) * 512],
                                         start=True, stop=True)
                    i = r - 32 - 4 * j
                    if i >= 0:
                        w = 128 * (i + 1)
                        for hl in range(2):
                            nc.vector.tensor_add(st[:, hl, 0:w], st[:, hl, 0:w],
                                                 msk_sb[:, i, 0:w])
                    pt = pt_pool.tile([128, 2, 512], BF16, tag="pt")
                    nc.scalar.activation(out=pt, in_=st,
                                         func=mybir.ActivationFunctionType.Exp,
                                         bias=zbias, scale=SCALE)
                    # PV emitted one r behind so it never waits on the exp
                    pv_q.append((r, pt))
                    if len(pv_q) == 2:
                        rr, ptp = pv_q.pop(0)
                        for hl in range(2):
                            nc.tensor.matmul(o_acc[:, hl, :], lhsT=v_sb[:, rr, :],
                                             rhs=ptp[:, hl, :],
                                             start=(rr == 0), stop=False)
                    pt_pair.append(pt)
                    if len(pt_pair) == 2:
                        u = tmp_pool.tile([128, 2, 512], BF16, tag="u", bufs=1)
                        nc.vector.tensor_add(u, pt_pair[0], pt_pair[1])
                        nc.vector.tensor_add(sacc, sacc, u)
                        pt_pair = []
                if pt_pair:
                    nc.vector.tensor_add(sacc, sacc, pt_pair[0])
                    pt_pair = []
                rr, ptp = pv_q.pop(0)
                for hl in range(2):
                    nc.tensor.matmul(o_acc[:, hl, :], lhsT=v_sb[:, rr, :],
                                     rhs=ptp[:, hl, :],
                                     start=(rr == 0), stop=True)
                # sweep end: free o_acc promptly with a scalar evac, then
                # partition-reduce via f32 ones-matmul, recip, normalize.
                oraw = sacc_pool.tile([128, 2, 512], F32, tag="oraw")
                nc.scalar.copy(out=oraw, in_=o_acc)
                saccb = stg_pool.tile([128, 2, 512], BF16, tag="saccb")
                nc.vector.tensor_copy(out=saccb, in_=sacc)
                dps = psumB.tile([128, 2, 512], F32, tag="scr", name="dps")
                for hl in range(2):
                    nc.tensor.matmul(dps[:, hl, :], lhsT=ones_b, rhs=saccb[:, hl, :],
                                     start=True, stop=True)
                pri = sacc_pool.tile([128, 2, 512], F32, tag="pri")
                nc.vector.reciprocal_approx_fast(out=pri, in_=dps)
                stg = stg_pool.tile([128, 2, 512], BF16, tag="stg")
                nc.vector.tensor_mul(stg, oraw, pri)
                nc.scalar.dma_start(out=ag_ins[hp][j][:], in_=stg.rearrange("p a b -> p (a b)"))
                nc.gpsimd.collective_compute(
                    "AllGather",
                    mybir.AluOpType.bypass,
                    ins=[ag_ins[hp][j][:]],
                    outs=[ag_outs[hp][j][:]],
                    replica_groups=ones_rg,
                )
            if j >= 1:
                emit_oproj(j - 1)
        emit_oproj(NQJ - 1)

    nc.compile()
    return nc


def _host_prep(context, query, w_qkv, w_o, q_norm_w, k_norm_w):
    context = np.asarray(context, dtype=np.float32)
    query = np.asarray(query, dtype=np.float32)
    w_qkv = np.asarray(w_qkv, dtype=np.float32)
    w_o = np.asarray(w_o, dtype=np.float32)
    q_norm_w = np.asarray(q_norm_w, dtype=np.float32)
    k_norm_w = np.asarray(k_norm_w, dtype=np.float32)

    ck = np.concatenate([context, query], axis=0)  # [KV, HID]
    ckT = np.ascontiguousarray(ck.T).astype(bfloat16)  # [HID, KV]

    wq = w_qkv[:, :H * D]
    wk = w_qkv[:, H * D:H * D + KVH * D]
    wv = w_qkv[:, H * D + KVH * D:]

    half = D // 2
    inv_freq = (1.0 / (THETA ** (np.arange(0, half, dtype=np.float32) / half))).astype(np.float32)
    pos = np.arange(KV, dtype=np.float32)
    freqs = pos[:, None] * inv_freq[None, :]   # [KV, 64]
    c = np.cos(freqs).T                        # [64, KV]
    s = np.sin(freqs).T

    def make_csT(nw):
        nw1 = nw[:half, None]
        nw2 = nw[half:, None]
        A = np.concatenate([c * nw1, c * nw2], axis=0)       # [128, KV]
        B = np.concatenate([-s * nw2, s * nw1], axis=0)      # [128, KV]
        return np.concatenate([A, B], axis=0).astype(bfloat16)  # [256, KV]

    cskT_full = make_csT(k_norm_w)
    csqT_full = make_csT(q_norm_w)[:, CTX:]

    p = np.arange(128)[:, None]
    q = np.arange(512)[None, :]
    msk = np.concatenate(
        [np.where(128 * i + p <= q, 0.0, MASKVAL) for i in range(4)],
        axis=1).astype(bfloat16)  # [128, 2048]

    rot = np.zeros((128, 128), dtype=np.float32)
    rot[(np.arange(128) + 64) % 128, np.arange(128)] = 1.0
    rot = rot.astype(bfloat16)

    in_maps = []
    for cidx in range(NCORES):
        in_maps.append({
            "ckT": ckT,
            "wq": np.ascontiguousarray(wq[:, cidx * HPC * D:(cidx + 1) * HPC * D]).astype(bfloat16),
            "wkv": np.ascontiguousarray(
                np.concatenate([wk[:, cidx * D:(cidx + 1) * D], wv[:, cidx * D:(cidx + 1) * D]], axis=1)
            ).astype(bfloat16),
            "wo": np.ascontiguousarray(w_o[:, cidx * HPC * D:(cidx + 1) * HPC * D]).astype(bfloat16),
            "csqT": csqT_full,
            "cskT": cskT_full,
            "msk": msk,
            "rot": rot,
        })
    return in_maps


def kernel(context, query, w_qkv, w_o, q_norm_w, k_norm_w, **kw):
    if "nc" not in _STATE:
        _STATE["nc"] = _build()
    nc = _STATE["nc"]
    in_maps = _host_prep(context, query, w_qkv, w_o, q_norm_w, k_norm_w)
    res = run_bass_kernel_spmd(nc, in_maps, list(range(NCORES)), **kw)
    out = np.concatenate([np.asarray(res.results[c]["out"]) for c in range(NCORES)], axis=1)
    if kw:
        return out.astype(np.float32), res
    return out.astype(np.float32)


# revision 24
# speedup vs baseline: 1.3520x; 1.0567x over previous
"""DFlash Qwen3 cross-attention on 8 TRN2 NeuronCores.

Sharding: tensor-parallel over heads. Core c owns KV head c (KVH=8) and the
4 query heads 4c..4c+3 of its GQA group.

v4 structure (evolved from baseline/v2/v3 trace analysis):
- All DMAs on HWDGE rings (sync = big streams + o_proj loads, scalar =
  cos/sin loads, AG staging, output stores).  No SWDGE descriptor-gen.
- Both projections are W-stationary: lhsT = weight chunk, moving = ckT
  columns at N=512, so Q^T and K^T come out of PSUM directly in the
  [d, pos] layout attention wants (no per-chunk PE transposes) and the
  KV matmul count halves vs the ck-stationary form.  V^T is transposed
  back to natural via 48 PE transposes.
- RMSNorm in transposed space: evac PSUM->bf16 SBUF, square (DVE 2x),
  partition-sum via a bf16 ones-matmul on the tensor engine (output is
  broadcast over partitions), ACT sqrt + reciprocal_approx_fast, rope
  via two host-precomputed transposed cos/sin tiles (norm weight folded
  in), final per-column 1/rms multiply writes qT/kT directly.
- Attention: j outermost (AG -> o_proj one j-tile behind), head pairs
  share kT/v stationaries, ONE [128,1024] exp per r-step, softmax
  denominator on vector only via bf16 pair-sums + f32 accumulate.
- Sweep end: partition-reduce of the denominator via an f32 ones-matmul
  (tensor), reciprocal_approx_fast, normalize straight out of PSUM.
  gpsimd runs ONLY the collective triggers: the AllGather trigger blocks
  its queue until the collective completes (~20us), which in v3 starved
  partition_all_reduce and stalled the whole pipeline at j boundaries.
"""

from contextlib import ExitStack

import numpy as np
from ml_dtypes import bfloat16

import concourse.bass as bass
import concourse.bass_isa as bass_isa
import concourse.mybir as mybir
import concourse.tile as tile
from concourse import bacc
from concourse.bass_utils import run_bass_kernel_spmd
from concourse.masks import make_identity

H = 32
KVH = 8
D = 128
HID = 4096
CTX = 4096
QL = 2048
KV = CTX + QL  # 6144
NCORES = 8
HPC = H // NCORES  # 4 q heads per core
THETA = 1000000.0
EPS = 1e-6
SCALE = float(D) ** -0.5

NHD = HID // 128  # 32 contraction chunks
NKV = KV // 128  # 48 kv chunks
NQC = QL // 128  # 16 q row chunks
NQJ = QL // 512  # 4 q column tiles for attention
MASKVAL = -1e6

F32 = mybir.dt.float32
BF16 = mybir.dt.bfloat16
MULT = mybir.AluOpType.mult

_STATE = {}


def _build():
    nc = bacc.Bacc()

    ckT = nc.declare_dram_parameter("ckT", [HID, KV], BF16, isOutput=False)
    wq = nc.declare_dram_parameter("wq", [HID, HPC * D], BF16, isOutput=False)
    wkv = nc.declare_dram_parameter("wkv", [HID, 2 * D], BF16, isOutput=False)
    wo = nc.declare_dram_parameter("wo", [HID, HPC * D], BF16, isOutput=False)
    csqT = nc.declare_dram_parameter("csqT", [2 * D, QL], BF16, isOutput=False)
    cskT = nc.declare_dram_parameter("cskT", [2 * D, KV], BF16, isOutput=False)
    msk = nc.declare_dram_parameter("msk", [128, 4 * 512], BF16, isOutput=False)
    rot = nc.declare_dram_parameter("rot", [128, 128], BF16, isOutput=False)
    out_ext = nc.declare_dram_parameter("out", [QL, HPC * D], F32, isOutput=True)

    ag_ins = [[nc.dram_tensor(f"ag_in{hp}_{j}", [128, 1024], BF16) for j in range(NQJ)]
              for hp in range(2)]
    ag_outs = [[nc.dram_tensor(f"ag_out{hp}_{j}", [NCORES * 128, 1024], BF16,
                               addr_space="Shared") for j in range(NQJ)]
               for hp in range(2)]

    with tile.TileContext(nc) as tc, ExitStack() as ctx:
        singles = ctx.enter_context(tc.tile_pool(name="singles", bufs=1))
        ck_pool = ctx.enter_context(tc.tile_pool(name="ckp", bufs=6))
        cs_pool = ctx.enter_context(tc.tile_pool(name="csp", bufs=2))
        evac_pool = ctx.enter_context(tc.tile_pool(name="evac", bufs=3))
        tmp_pool = ctx.enter_context(tc.tile_pool(name="tmp", bufs=2))
        pt_pool = ctx.enter_context(tc.tile_pool(name="ptp", bufs=4))
        sacc_pool = ctx.enter_context(tc.tile_pool(name="sacc", bufs=1))
        stg_pool = ctx.enter_context(tc.tile_pool(name="stg", bufs=2))
        at_pool = ctx.enter_context(tc.tile_pool(name="atp", bufs=2))
        # PSUM: A = proj accumulators / attention S^T (2 x 2 banks = 4);
        # B: "oacc" (2 banks) + "scr" (2 banks: proj norm-sums + V
        # transposes, attention denominator, o_proj accumulator)
        psumA = ctx.enter_context(tc.tile_pool(name="psA", bufs=2, space="PSUM"))
        psumB = ctx.enter_context(tc.tile_pool(name="psB", bufs=1, space="PSUM"))

        # ---- resident tensors ----
        wq_sb = singles.tile([128, NHD, HPC * D], BF16)
        for wpc in range(8):
            kk = NHD // 8
            nc.scalar.dma_start(
                out=wq_sb[:, wpc * kk:(wpc + 1) * kk, :],
                in_=wq[wpc * kk * 128:(wpc + 1) * kk * 128, :].rearrange(
                    "(k p) n -> p k n", p=128))
        wkv_sb = singles.tile([128, NHD, 2 * D], BF16)
        for wpc in range(2):
            kk = NHD // 2
            nc.scalar.dma_start(
                out=wkv_sb[:, wpc * kk:(wpc + 1) * kk, :],
                in_=wkv[wpc * kk * 128:(wpc + 1) * kk * 128, :].rearrange(
                    "(k p) n -> p k n", p=128))
        wo_sb = singles.tile([128, NHD, HPC * D], BF16)
        nc.scalar.dma_start(out=wo_sb[:], in_=wo[:, :].rearrange("(k p) n -> p k n", p=128))
        msk_sb = singles.tile([128, 4, 512], BF16)
        nc.scalar.dma_start(out=msk_sb[:], in_=msk[:, :].rearrange("p (i c) -> p i c", i=4))

        identb = singles.tile([128, 128], BF16)
        make_identity(nc, identb)
        rot_sb = singles.tile([128, 128], BF16)
        nc.scalar.dma_start(out=rot_sb, in_=rot[:, :])
        ones_b = singles.tile([128, 128], BF16)
        nc.vector.memset(ones_b, 1.0)
        ones_f = singles.tile([128, 128], F32)
        nc.vector.memset(ones_f, 1.0)
        epst = singles.tile([128, 1], F32)
        nc.vector.memset(epst, EPS)
        zbias = singles.tile([128, 1], F32)
        nc.vector.memset(zbias, 0.0)

        qT_sb = singles.tile([128, HPC, QL], BF16)  # Q^T per head: [d, h, q]
        kT_sb = singles.tile([128, KV], BF16)  # K^T: [d, kv]
        v_sb = singles.tile([128, NKV, D], BF16)  # V: [kv%128, r, d]

        pend1 = [None]
        pend2 = [None]

        def step_pipeline(new_stage1=None):
            if pend2[0] is not None:
                pend2[0]()
            pend2[0] = None
            if pend1[0] is not None:
                pend2[0] = pend1[0]()
            pend1[0] = new_stage1

        def norm_rope_T(xb, cst, out_slice):
            """Transposed-space rmsnorm+rope for one [128, 512] tile.

            xb: [128, 512] bf16 (rows = d, cols = positions).
            cst: [128, 2, 512] bf16 (A_T, B_T with norm weight folded in).
            The d -> (d+64)%128 partition rotation the rope needs is done
            with a permutation matmul (DVE lanes cannot shift partitions).
            Writes normalized+roped bf16 into out_slice ([128, 512])."""
            sq = tmp_pool.tile([128, 512], BF16, tag="sq")
            nc.vector.tensor_mul(sq, xb, xb)
            scr = psumB.tile([128, 2, 512], F32, tag="scr", name="normscr")
            nc.tensor.matmul(scr[:, 0, :], lhsT=ones_b, rhs=sq, start=True, stop=True)
            nc.tensor.matmul(scr[:, 1, :], lhsT=rot_sb, rhs=xb, start=True, stop=True)
            sqr = tmp_pool.tile([128, 512], F32, tag="sqr", bufs=1)
            nc.scalar.activation(out=sqr, in_=scr[:, 0, :],
                                 func=mybir.ActivationFunctionType.Sqrt,
                                 bias=epst, scale=1.0 / D)
            rs = tmp_pool.tile([128, 512], F32, tag="rs", bufs=1)
            nc.vector.reciprocal_approx_fast(out=rs, in_=sqr)
            t1 = tmp_pool.tile([128, 512], BF16, tag="t1")
            nc.vector.tensor_mul(t1, xb, cst[:, 0, :])
            t2 = tmp_pool.tile([128, 512], BF16, tag="t2")
            nc.vector.tensor_mul(t2, scr[:, 1, :], cst[:, 1, :])
            rsum = tmp_pool.tile([128, 512], BF16, tag="rsum")
            nc.vector.tensor_add(rsum, t1, t2)
            nc.vector.tensor_mul(out_slice, rsum, rs)

        # ================= Q projection =================
        # W-stationary: psum tile [:, h%2, :] = qT of head h for this
        # 512-column group of q positions.
        def q_stage1(g, pq01):
            qb = evac_pool.tile([128, 4, 512], BF16, tag="qb", bufs=2)
            nc.scalar.copy(out=qb[:, 0:2, :], in_=pq01[0])
            nc.scalar.copy(out=qb[:, 2:4, :], in_=pq01[1])
            cst = cs_pool.tile([128, 2, 512], BF16, tag="cs")
            nc.scalar.dma_start(out=cst, in_=csqT[:, g * 512:(g + 1) * 512].rearrange(
                "(two p) c -> p two c", p=128))
            for h in range(HPC):
                norm_rope_T(qb[:, h, :], cst,
                            qT_sb[:, h, g * 512:(g + 1) * 512])
            return None

        for g in range(4):
            pq01 = [psumA.tile([128, 2, 512], F32, tag="acc", name=f"pq{g}_{i}")
                    for i in range(2)]
            for k2 in range(NHD // 2):
                ckq = ck_pool.tile([128, 2, 512], BF16, tag="ck")
                nc.sync.dma_start(
                    out=ckq,
                    in_=ckT[k2 * 256:(k2 + 1) * 256,
                            CTX + g * 512: CTX + (g + 1) * 512].rearrange(
                        "(two p) c -> p two c", p=128))
                for two in range(2):
                    k = 2 * k2 + two
                    for h in range(HPC):
                        nc.tensor.matmul(pq01[h // 2][:, h % 2, :],
                                         lhsT=wq_sb[:, k, h * 128:(h + 1) * 128],
                                         rhs=ckq[:, two, :], start=(k == 0), stop=(k == NHD - 1))
            step_pipeline(lambda g=g, pq01=pq01: q_stage1(g, pq01))

        # ================= K/V projection =================
        # W-stationary: pk[:, 0, :] = K^T, pk[:, 1, :] = V^T for this
        # 512-column group of kv positions.
        def kv_stage1(rq, pk):
            kb = evac_pool.tile([128, 2, 512], BF16, tag="kb")
            nc.scalar.copy(out=kb, in_=pk)
            cst = cs_pool.tile([128, 2, 512], BF16, tag="cs")
            nc.scalar.dma_start(out=cst, in_=cskT[:, rq * 512:(rq + 1) * 512].rearrange(
                "(two p) c -> p two c", p=128))
            norm_rope_T(kb[:, 0, :], cst, kT_sb[:, rq * 512:(rq + 1) * 512])

            def kv_stage2(rq=rq, kb=kb):
                tpv = psumB.tile([128, 512], BF16, tag="scr", name="tpv")
                for rr in range(4):
                    nc.tensor.transpose(tpv[:, rr * 128:(rr + 1) * 128],
                                        kb[:, 1, rr * 128:(rr + 1) * 128], identb)
                nc.scalar.copy(out=v_sb[:, rq * 4:(rq + 1) * 4, :],
                               in_=tpv.rearrange("p (r d) -> p r d", r=4))
            return kv_stage2

        for rq in range(NKV // 4):
            pk = psumA.tile([128, 2, 512], F32, tag="acc", name=f"pk{rq}")
            for k2 in range(NHD // 2):
                ckt = ck_pool.tile([128, 2, 512], BF16, tag="ck")
                nc.sync.dma_start(
                    out=ckt,
                    in_=ckT[k2 * 256:(k2 + 1) * 256,
                            rq * 512:(rq + 1) * 512].rearrange("(two p) c -> p two c", p=128))
                for two in range(2):
                    k = 2 * k2 + two
                    for half in range(2):
                        nc.tensor.matmul(pk[:, half, :],
                                         lhsT=wkv_sb[:, k, half * 128:(half + 1) * 128],
                                         rhs=ckt[:, two, :], start=(k == 0), stop=(k == NHD - 1))
            step_pipeline(lambda rq=rq, pk=pk: kv_stage1(rq, pk))

        step_pipeline()
        step_pipeline()

        # ================= attention =================
        ones_rg = [list(range(NCORES))]

        def emit_oproj(jq, final=False):
            if not final:
                for qc in range(4 * jq, 4 * jq + 4):
                    qo = (qc % 4) * 128
                    po = psumB.tile([128, 512], F32, tag="scr", name="po")
                    first = True
                    for hp in range(2):
                        for hl in range(2):
                            at = at_pool.tile([128, NCORES, 128], BF16, tag="at")
                            nc.sync.dma_start(
                                out=at,
                                in_=ag_outs[hp][jq][:, hl * 512 + qo: hl * 512 + qo + 128]
                                .rearrange("(c p) q -> p c q", p=128))
                            for ci in range(NCORES):
                                nc.tensor.matmul(po, lhsT=at[:, ci, :],
                                                 rhs=wo_sb[:, HPC * ci + 2 * hp + hl, :],
                                                 start=first,
                                                 stop=(hp == 1 and hl == 1 and ci == NCORES - 1))
                                first = False
                    ot = stg_pool.tile([128, 512], F32, tag="ot")
                    nc.vector.tensor_copy(out=ot, in_=po)
                    nc.scalar.dma_start(out=out_ext[qc * 128:(qc + 1) * 128, :], in_=ot)
            else:
                # final j-tile: 4 po accumulators in the freed attention PSUM
                # ring; hp0 (whose AllGather landed a sweep ago) fully
                # consumed before the hp1 matmuls that wait on the last AG.
                pof = [psumA.tile([128, 2, 512], F32, tag="acc", name=f"pofin{i}")
                       for i in range(2)]
                for hp in range(2):
                    for hl in range(2):
                        for qc4 in range(4):
                            qc = 4 * jq + qc4
                            qo = (qc % 4) * 128
                            at = at_pool.tile([128, NCORES, 128], BF16, tag="at")
                            nc.sync.dma_start(
                                out=at,
                                in_=ag_outs[hp][jq][:, hl * 512 + qo: hl * 512 + qo + 128]
                                .rearrange("(c p) q -> p c q", p=128))
                            for ci in range(NCORES):
                                nc.tensor.matmul(pof[qc4 // 2][:, qc4 % 2, :],
                                                 lhsT=at[:, ci, :],
                                                 rhs=wo_sb[:, HPC * ci + 2 * hp + hl, :],
                                                 start=(hp == 0 and hl == 0 and ci == 0),
                                                 stop=(hp == 1 and hl == 1 and ci == NCORES - 1))
                for qc4 in range(4):
                    qc = 4 * jq + qc4
                    ot = stg_pool.tile([128, 512], F32, tag="ot")
                    nc.vector.tensor_copy(out=ot, in_=pof[qc4 // 2][:, qc4 % 2, :])
                    nc.scalar.dma_start(out=out_ext[qc * 128:(qc + 1) * 128, :], in_=ot)

        for j in range(NQJ):
            rmax = 35 + 4 * j if j < NQJ - 1 else NKV - 1
            for hp in range(2):
                sacc = sacc_pool.tile([128, 2, 512], F32, tag="sacc", bufs=2)
                nc.vector.memset(sacc, 0.0)
                o_acc = psumB.tile([128, 2, 512], F32, tag="oacc")
                pt_pair = []
                pv_q = []
                for r in range(rmax + 1):
                    st = psumA.tile([128, 2, 512], F32, tag="acc")
                    for hl in range(2):
                        nc.tensor.matmul(st[:, hl, :], lhsT=kT_sb[:, r * 128:(r + 1) * 128],
                                         rhs=qT_sb[:, 2 * hp + hl, j * 512:(j + 1) * 512],
                                         start=True, stop=True)
                    i = r - 32 - 4 * j
                    if i >= 0:
                        w = 128 * (i + 1)
                        for hl in range(2):
                            nc.vector.tensor_add(st[:, hl, 0:w], st[:, hl, 0:w],
                                                 msk_sb[:, i, 0:w])
                    pt = pt_pool.tile([128, 2, 512], BF16, tag="pt")
                    nc.scalar.activation(out=pt, in_=st,
                                         func=mybir.ActivationFunctionType.Exp,
                                         bias=zbias, scale=SCALE)
                    # PV emitted one r behind so it never waits on the exp
                    pv_q.append((r, pt))
                    if len(pv_q) == 2:
                        rr, ptp = pv_q.pop(0)
                        for hl in range(2):
                            nc.tensor.matmul(o_acc[:, hl, :], lhsT=v_sb[:, rr, :],
                                             rhs=ptp[:, hl, :],
                                             start=(rr == 0), stop=False)
                    pt_pair.append(pt)
                    if len(pt_pair) == 2:
                        u = tmp_pool.tile([128, 2, 512], BF16, tag="u", bufs=1)
                        nc.vector.tensor_add(u, pt_pair[0], pt_pair[1])
                        nc.vector.tensor_add(sacc, sacc, u)
                        pt_pair = []
                if pt_pair:
                    nc.vector.tensor_add(sacc, sacc, pt_pair[0])
                    pt_pair = []
                rr, ptp = pv_q.pop(0)
                for hl in range(2):
                    nc.tensor.matmul(o_acc[:, hl, :], lhsT=v_sb[:, rr, :],
                                     rhs=ptp[:, hl, :],
                                     start=(rr == 0), stop=True)
                # sweep end: free o_acc promptly with a scalar evac, then
                # partition-reduce via f32 ones-matmul, recip, normalize.
                oraw = sacc_pool.tile([128, 2, 512], F32, tag="oraw")
                nc.scalar.copy(out=oraw, in_=o_acc)
                saccb = stg_pool.tile([128, 2, 512], BF16, tag="saccb", bufs=1)
                nc.vector.tensor_copy(out=saccb, in_=sacc)
                dps = psumB.tile([128, 2, 512], F32, tag="scr", name="dps")
                for hl in range(2):
                    nc.tensor.matmul(dps[:, hl, :], lhsT=ones_b, rhs=saccb[:, hl, :],
                                     start=True, stop=True)
                pri = sacc_pool.tile([128, 2, 512], F32, tag="pri")
                nc.vector.reciprocal_approx_fast(out=pri, in_=dps)
                stg = stg_pool.tile([128, 2, 512], BF16, tag="stg")
                nc.vector.tensor_mul(stg, oraw, pri)
                nc.scalar.dma_start(out=ag_ins[hp][j][:], in_=stg.rearrange("p a b -> p (a b)"))
                nc.gpsimd.collective_compute(
                    "AllGather",
                    mybir.AluOpType.bypass,
                    ins=[ag_ins[hp][j][:]],
                    outs=[ag_outs[hp][j][:]],
                    replica_groups=ones_rg,
                )
            if j >= 1:
                emit_oproj(j - 1)
        emit_oproj(NQJ - 1, final=True)

    nc.compile()
    return nc


def _host_prep(context, query, w_qkv, w_o, q_norm_w, k_norm_w):
    context = np.asarray(context, dtype=np.float32)
    query = np.asarray(query, dtype=np.float32)
    w_qkv = np.asarray(w_qkv, dtype=np.float32)
    w_o = np.asarray(w_o, dtype=np.float32)
    q_norm_w = np.asarray(q_norm_w, dtype=np.float32)
    k_norm_w = np.asarray(k_norm_w, dtype=np.float32)

    ck = np.concatenate([context, query], axis=0)  # [KV, HID]
    ckT = np.ascontiguousarray(ck.T).astype(bfloat16)  # [HID, KV]

    wq = w_qkv[:, :H * D]
    wk = w_qkv[:, H * D:H * D + KVH * D]
    wv = w_qkv[:, H * D + KVH * D:]

    half = D // 2
    inv_freq = (1.0 / (THETA ** (np.arange(0, half, dtype=np.float32) / half))).astype(np.float32)
    pos = np.arange(KV, dtype=np.float32)
    freqs = pos[:, None] * inv_freq[None, :]   # [KV, 64]
    c = np.cos(freqs).T                        # [64, KV]
    s = np.sin(freqs).T

    def make_csT(nw):
        nw1 = nw[:half, None]
        nw2 = nw[half:, None]
        A = np.concatenate([c * nw1, c * nw2], axis=0)       # [128, KV]
        B = np.concatenate([-s * nw2, s * nw1], axis=0)      # [128, KV]
        return np.concatenate([A, B], axis=0).astype(bfloat16)  # [256, KV]

    cskT_full = make_csT(k_norm_w)
    csqT_full = make_csT(q_norm_w)[:, CTX:]

    p = np.arange(128)[:, None]
    q = np.arange(512)[None, :]
    msk = np.concatenate(
        [np.where(128 * i + p <= q, 0.0, MASKVAL) for i in range(4)],
        axis=1).astype(bfloat16)  # [128, 2048]

    rot = np.zeros((128, 128), dtype=np.float32)
    rot[(np.arange(128) + 64) % 128, np.arange(128)] = 1.0
    rot = rot.astype(bfloat16)

    in_maps = []
    for cidx in range(NCORES):
        in_maps.append({
            "ckT": ckT,
            "wq": np.ascontiguousarray(wq[:, cidx * HPC * D:(cidx + 1) * HPC * D]).astype(bfloat16),
            "wkv": np.ascontiguousarray(
                np.concatenate([wk[:, cidx * D:(cidx + 1) * D], wv[:, cidx * D:(cidx + 1) * D]], axis=1)
            ).astype(bfloat16),
            "wo": np.ascontiguousarray(w_o[:, cidx * HPC * D:(cidx + 1) * HPC * D]).astype(bfloat16),
            "csqT": csqT_full,
            "cskT": cskT_full,
            "msk": msk,
            "rot": rot,
        })
    return in_maps


def kernel(context, query, w_qkv, w_o, q_norm_w, k_norm_w, **kw):
    if "nc" not in _STATE:
        _STATE["nc"] = _build()
    nc = _STATE["nc"]
    in_maps = _host_prep(context, query, w_qkv, w_o, q_norm_w, k_norm_w)
    res = run_bass_kernel_spmd(nc, in_maps, list(range(NCORES)), **kw)
    out = np.concatenate([np.asarray(res.results[c]["out"]) for c in range(NCORES)], axis=1)
    if kw:
        return out.astype(np.float32), res
    return out.astype(np.float32)
